# revision 9
# baseline (speedup 1.0000x reference)
"""Trainium2 Bass kernel for nn_BenchCADecoder (cellular-automaton decoder).

Model: x = embed[tokens]+pos; rw = softmax(gate*1e-3 @ sel_w + sel_b) (step
invariant); 5 CA steps of x = LN(x + sum_r rw[t,r] * MLP_r([x, roll(x,1),
roll(x,-1)])); out = LN_f(x) @ head_w.

Sharding: pure data-parallel over (batch, T-half): 8 cores x 1024 tokens,
each with a circular halo of 5 tokens per side so the 5 neighbor-coupled
steps need zero inter-core communication (window shrinks by 1/side/step).

On-chip layout: x kept transposed [D, tok] so roll() is a free-dim slice
shift and both MLP matmuls run with the contraction on partitions. All MLP
and head matmuls in bf16 (weights pre-cast on host, x cast on-chip once per
step): bf16 enables fast-weight-load so the per-matmul LDWEIGHTS hides
under the previous matmul's stream, and it halves weight DMA. The residual
x stream and all LayerNorm math stay f32. The rule-weighted sum is folded
into PSUM accumulation of the second matmul by pre-scaling gelu outputs
with broadcast rule weights. LayerNorm stats (partition-dim reductions) via
ones-vector matmuls in f32r. Head output is written bf16 and widened to
f32 on host (halves the 131MB/core logit write).
"""

import sys
from contextlib import ExitStack

import numpy as np

sys.path.insert(0, "/opt/trn_rl_repo")

import concourse.bacc as bacc
import concourse.bass as bass
import concourse.mybir as mybir
import concourse.tile as tile
from concourse.bass import IndirectOffsetOnAxis
from concourse.bass_utils import run_bass_kernel_spmd
from concourse.masks import make_identity

F32 = mybir.dt.float32
F32R = mybir.dt.float32r
BF16 = mybir.dt.bfloat16
I32 = mybir.dt.int32
AF = mybir.ActivationFunctionType
OP = mybir.AluOpType

P = 128


class Cfg:
    def __init__(self, D=512, R=8, V=32000, T=2048, B=4, steps=5, own=1024,
                 halo=5, eps=1e-5, gate_scale=1e-3, newton=True):
        self.D, self.R, self.V, self.T, self.B = D, R, V, T, B
        self.steps, self.own, self.halo = steps, own, halo
        self.eps, self.gate_scale = eps, gate_scale
        self.newton = newton
        self.DC = D // P                 # d chunks
        self.HID = 2 * D
        self.HC = self.HID // P          # hidden chunks
        self.KC = 3 * self.DC            # contraction chunks for mm1
        self.WIN = own + 2 * halo        # gathered token window (1034)
        self.NT9 = (self.WIN + P - 1) // P   # gather tiles (9)
        self.WBUF = self.NT9 * P         # x buffer cols (1152)
        self.own_col0 = halo             # first owned col in x buffer
        self.n_tok_chunks = own // P     # head token chunks (8 x 128)
        # head vocab tiling: 16 groups x 4 tiles x 500 cols = 32000
        self.VGW = 500
        self.VPG = 4
        self.NVG = V // (self.VGW * self.VPG)

    def step_tiles(self, s):
        """Output-window tiles for CA step s: [(col0, width)] x3."""
        W = self.own + 2 * (self.steps - 1 - s)
        lo = s + 1
        return self._split3(lo, W)

    def final_tiles(self):
        return self._split3(self.own_col0, self.own)

    @staticmethod
    def _split3(lo, W):
        # three even-width tiles (f32r matmuls require even free sizes)
        w = ((W + 5) // 6) * 2
        return [(lo, w), (lo + w, w), (lo + 2 * w, W - 2 * w)]


def _r(ap):
    """View an f32 AP as float32r for PE consumption."""
    return ap.bitcast(F32R)


def build_nc(cfg: Cfg, num_devices=8):
    """Build the single-core (SPMD) Bass module."""
    nc = bacc.Bacc("TRN2", target_bir_lowering=False, debug=False,
                   num_devices=num_devices)
    D, R, V, HC, KC = cfg.D, cfg.R, cfg.V, cfg.HC, cfg.KC
    S = cfg.steps

    # ---- DRAM I/O ------------------------------------------------------
    toksT = nc.dram_tensor("toksT", [P, cfg.NT9], I32, kind="ExternalInput").ap()
    gate = nc.dram_tensor("gate", [cfg.WBUF, D], F32, kind="ExternalInput").ap()
    pos = nc.dram_tensor("pos", [cfg.WBUF, D], F32, kind="ExternalInput").ap()
    embed = nc.dram_tensor("embed", [V, D], F32, kind="ExternalInput").ap()
    w1t = nc.dram_tensor("w1t", [R, HC, P, KC, P], BF16, kind="ExternalInput").ap()
    b1 = nc.dram_tensor("b1", [R, 2 * D], F32, kind="ExternalInput").ap()
    w2t = nc.dram_tensor("w2t", [R, P, HC, cfg.DC, P], BF16,
                         kind="ExternalInput").ap()
    b2 = nc.dram_tensor("b2", [R, D], BF16, kind="ExternalInput").ap()
    selw = nc.dram_tensor("selw", [D, R], F32R, kind="ExternalInput").ap()
    selb = nc.dram_tensor("selb", [1, R], F32R, kind="ExternalInput").ap()
    ng = nc.dram_tensor("ng", [S, D], F32, kind="ExternalInput").ap()
    nb_ = nc.dram_tensor("nb", [S, D], F32, kind="ExternalInput").ap()
    lg = nc.dram_tensor("lg", [1, D], F32, kind="ExternalInput").ap()
    lb = nc.dram_tensor("lb", [1, D], F32, kind="ExternalInput").ap()
    headw = nc.dram_tensor("headw", [D, V], BF16, kind="ExternalInput").ap()
    ones1_d = nc.dram_tensor("ones1", [P, 1], F32R, kind="ExternalInput").ap()
    ones8_d = nc.dram_tensor("ones8d", [8, P], F32R, kind="ExternalInput").ap()
    out = nc.dram_tensor("out", [cfg.own, V], BF16, kind="ExternalOutput").ap()

    with ExitStack() as ctx:
        ctx.enter_context(nc.allow_low_precision(reason="bf16 mms by design"))
        tc = ctx.enter_context(tile.TileContext(nc))
        _emit(ctx, tc, cfg, toksT, gate, pos, embed, w1t, b1, w2t, b2, selw,
              selb, ng, nb_, lg, lb, headw, out, ones1_d, ones8_d)
    nc.compile()
    return nc


def _emit(ctx, tc, cfg, toksT, gate, pos, embed, w1t, b1, w2t, b2, selw,
          selb, ng, nb_, lg, lb, headw, out, ones1_d, ones8_d):
    nc = tc.nc
    D, R, DC, HC, KC = cfg.D, cfg.R, cfg.DC, cfg.HC, cfg.KC
    S, WBUF, NT9 = cfg.steps, cfg.WBUF, cfg.NT9

    def mmr(o, lh, rh, start, stop):
        nc.tensor.matmul(o, _r(lh), _r(rh), start=start, stop=stop)

    def mmb(o, lh, rh, start, stop):
        nc.tensor.matmul(o, lh, rh, start=start, stop=stop)

    # ---- persistent SBUF ----------------------------------------------
    persist = ctx.enter_context(tc.tile_pool(name="persist", bufs=1))
    xA = persist.tile([P, DC, WBUF], F32R, name="xA")
    xB = persist.tile([P, DC, WBUF], F32R, name="xB")
    xb16 = persist.tile([P, DC, WBUF], BF16, name="xb16")
    rwB = persist.tile([P, R, WBUF], BF16, name="rwB")   # bcast rule weights
    rwT = persist.tile([R, WBUF], BF16, name="rwT")      # rw [r, tok]
    ident = persist.tile([P, P], F32, name="ident")
    ones128 = persist.tile([P, 1], F32R, name="ones128")
    ones8 = persist.tile([8, P], F32R, name="ones8")
    b1_sb = persist.tile([P, R, HC], F32, name="b1_sb")
    b2_sb = persist.tile([R, DC, P], BF16, name="b2_sb")
    ng_sb = persist.tile([P, S, DC], F32, name="ng_sb")
    nbv_sb = persist.tile([P, S, DC], F32, name="nbv_sb")
    lg_sb = persist.tile([P, 1, DC], F32, name="lg_sb")
    lb_sb = persist.tile([P, 1, DC], F32, name="lb_sb")

    make_identity(nc, ident)
    nc.sync.dma_start(out=ones128, in_=ones1_d)
    nc.sync.dma_start(out=ones8[0:8, :], in_=ones8_d)

    nc.sync.dma_start(out=b1_sb, in_=bass.AP(
        b1.tensor, 0, [[1, P], [2 * D, R], [P, HC]]))
    nc.sync.dma_start(out=b2_sb, in_=bass.AP(
        b2.tensor, 0, [[D, R], [P, DC], [1, P]]))
    nc.sync.dma_start(out=ng_sb, in_=bass.AP(
        ng.tensor, 0, [[1, P], [D, S], [P, DC]]))
    nc.sync.dma_start(out=nbv_sb, in_=bass.AP(
        nb_.tensor, 0, [[1, P], [D, S], [P, DC]]))
    nc.sync.dma_start(out=lg_sb, in_=bass.AP(
        lg.tensor, 0, [[1, P], [D, 1], [P, DC]]))
    nc.sync.dma_start(out=lb_sb, in_=bass.AP(
        lb.tensor, 0, [[1, P], [D, 1], [P, DC]]))

    # ---- setup: embed gather + pos -> xA/xb16; gate -> rw --------------
    with tc.tile_pool(name="setup", bufs=3) as sp, \
         tc.tile_pool(name="setup_ps", bufs=2, space="PSUM") as spp, \
         tc.tile_pool(name="setup_small", bufs=2) as ss:
        idx = persist.tile([P, NT9], I32, name="idx")
        nc.sync.dma_start(out=idx, in_=toksT)
        selw_sb = persist.tile([P, DC, R], F32R, name="selw_sb")
        nc.sync.dma_start(out=selw_sb, in_=bass.AP(
            selw.tensor, 0, [[R, P], [P * R, DC], [1, R]]))
        nc.vector.tensor_scalar_mul(selw_sb, selw_sb, cfg.gate_scale)
        selb_sb = persist.tile([1, R], F32R, name="selb_sb")
        nc.sync.dma_start(out=selb_sb, in_=selb)

        for i in range(NT9):
            # x tile: gather embed rows + pos
            xg = sp.tile([P, D], F32, tag="xg")
            nc.gpsimd.indirect_dma_start(
                out=xg, out_offset=None, in_=embed,
                in_offset=IndirectOffsetOnAxis(ap=idx[:, i:i + 1], axis=0))
            pt = sp.tile([P, D], F32, tag="pt")
            nc.sync.dma_start(out=pt, in_=pos[i * P:(i + 1) * P, :])
            nc.vector.tensor_add(xg, xg, pt)
            tp = spp.tile([P, DC, P], F32, space="PSUM", tag="tp")
            for dc in range(DC):
                nc.tensor.transpose(tp[:, dc, :], xg[:, dc * P:(dc + 1) * P], ident)
            nc.vector.tensor_copy(xA[:, :, i * P:(i + 1) * P], tp)
            nc.scalar.copy(xb16[:, :, i * P:(i + 1) * P], tp)

            # gate tile -> gateT (transposed), then logits -> rw
            gt = sp.tile([P, D], F32, tag="gt")
            nc.sync.dma_start(out=gt, in_=gate[i * P:(i + 1) * P, :])
            tg = spp.tile([P, DC, P], F32, space="PSUM", tag="tp")
            for dc in range(DC):
                nc.tensor.transpose(tg[:, dc, :], gt[:, dc * P:(dc + 1) * P], ident)
            gT = sp.tile([P, DC, P], F32R, tag="gT")
            nc.vector.tensor_copy(gT, tg)

            lp = spp.tile([P, R], F32, space="PSUM", tag="lp")
            for dc in range(DC):
                mmr(lp, gT[:, dc, :], selw_sb[:, dc, :], dc == 0, False)
            mmr(lp, ones8[0:1, :], selb_sb, False, True)  # rank-1 +sel_b
            e = ss.tile([P, R], F32, tag="e")
            nc.scalar.activation(e, lp, AF.Exp)
            esum = ss.tile([P, 1], F32, tag="es")
            nc.vector.tensor_reduce(esum, e, mybir.AxisListType.X, OP.add)
            nc.vector.reciprocal(esum, esum)
            nc.vector.tensor_scalar(out=e, in0=e, scalar1=esum, scalar2=None,
                                    op0=OP.mult)
            rp = spp.tile([R, P], F32, space="PSUM", tag="rp")
            nc.tensor.transpose(rp, e, ident)
            nc.vector.tensor_copy(rwT[:, i * P:(i + 1) * P], rp)

        # broadcast rw rows across partitions: rwB[p, r, c] = rw[tok c, r]
        # (bounce via DRAM: SBUF sources cannot have partition step 0)
        rw_dram = nc.dram_tensor("rw_scratch", [R, WBUF], BF16).ap()
        nc.sync.dma_start(out=rw_dram, in_=rwT)
        for r in range(R):
            nc.sync.dma_start(
                out=rwB[:, r, :],
                in_=bass.AP(rw_dram.tensor, r * WBUF, [[0, P], [1, WBUF]]))

    # ---- CA steps ------------------------------------------------------
    with tc.tile_pool(name="w1p", bufs=3) as wp, \
         tc.tile_pool(name="w2p", bufs=2) as w2p, \
         tc.tile_pool(name="g8p", bufs=2) as g8p, \
         tc.tile_pool(name="evsp", bufs=1) as evsp, \
         tc.tile_pool(name="rbp", bufs=1) as rp_, \
         tc.tile_pool(name="rowp", bufs=1) as rowp, \
         tc.tile_pool(name="evp", bufs=1, space="PSUM") as evp, \
         tc.tile_pool(name="hpp", bufs=2, space="PSUM") as hpp, \
         tc.tile_pool(name="stp", bufs=1, space="PSUM") as stp:

        def layer_norm(xc, xn, c0, nt, ev, g_col, b_col, bf16_only=False,
                       write_bf16=True):
            """LN of (xc[:, :, c0:c0+nt] + ev) -> xn cols (+ xb16 cast).

            ev may be None (final LN). bf16_only: write only xb16.
            """
            inv_d = 1.0 / D
            if ev is not None:
                rb = rp_.tile([P, DC, nt], F32R, tag="rb")
                nc.vector.tensor_add(rb, xc[:, :, c0:c0 + nt], ev)
            else:
                rb = xc[:, :, c0:c0 + nt]
            sq = rp_.tile([P, DC, nt], F32R, tag="sq")
            nc.scalar.square(sq, rb)
            st_s = stp.tile([1, 512], F32, space="PSUM", tag="sts")
            st_q = stp.tile([1, 512], F32, space="PSUM", tag="stq")
            for dc in range(DC):
                nc.tensor.matmul(st_s[:, :nt], ones128, rb[:, dc, :],
                                 start=dc == 0, stop=dc == DC - 1)
            for dc in range(DC):
                nc.tensor.matmul(st_q[:, :nt], ones128, sq[:, dc, :],
                                 start=dc == 0, stop=dc == DC - 1)
            mrow = rowp.tile([1, nt], F32, tag="mrow")
            nc.vector.tensor_scalar_mul(mrow, st_s[:, :nt], inv_d)
            msq = rowp.tile([1, nt], F32, tag="msq")
            nc.vector.tensor_mul(msq, mrow, mrow)
            wrow = rowp.tile([1, nt], F32, tag="wrow")
            # wrow = st_q/D - m^2 (+eps)
            nc.vector.scalar_tensor_tensor(out=wrow, in0=st_q[:, :nt],
                                           scalar=inv_d, in1=msq,
                                           op0=OP.mult, op1=OP.subtract)
            nc.vector.tensor_scalar_add(wrow, wrow, cfg.eps)
            srow = rowp.tile([1, nt], F32R, tag="srow")
            nc.scalar.activation(srow, wrow, AF.Sqrt)
            nc.vector.reciprocal(srow, srow)
            if cfg.newton:  # one Newton step: s *= 1.5 - 0.5*w*s*s
                t1 = rowp.tile([1, nt], F32, tag="msq", name="t1")
                nc.vector.tensor_mul(t1, wrow, srow)
                nc.vector.tensor_mul(t1, t1, srow)
                nc.vector.tensor_scalar(out=t1, in0=t1, scalar1=-0.5,
                                        scalar2=1.5, op0=OP.mult, op1=OP.add)
                nc.vector.tensor_mul(srow, srow, t1)
            # nms = -m*s
            nms = rowp.tile([1, nt], F32R, tag="nms")
            nc.vector.scalar_tensor_tensor(out=nms, in0=mrow, scalar=-1.0,
                                           in1=srow, op0=OP.mult, op1=OP.mult)
            bc = evp.tile([P, 2, 512], F32, space="PSUM", tag="ev")
            nc.tensor.matmul(bc[:, 0, :nt], ones8[0:1, :], srow,
                             start=True, stop=True)
            nc.tensor.matmul(bc[:, 1, :nt], ones8[0:1, :], nms,
                             start=True, stop=True)
            u = rp_.tile([P, DC, nt], F32, tag="sq", name="u")
            nc.vector.tensor_mul(u, rb,
                                 bc[:, 0:1, :nt].broadcast_to([P, DC, nt]))
            nc.vector.tensor_add(u, u,
                                 bc[:, 1:2, :nt].broadcast_to([P, DC, nt]))
            for dc in range(DC):
                dst = xb16 if bf16_only else xn
                nc.vector.tensor_scalar(
                    out=dst[:, dc, c0:c0 + nt], in0=u[:, dc, :],
                    scalar1=g_col[:, dc:dc + 1], scalar2=b_col[:, dc:dc + 1],
                    op0=OP.mult, op1=OP.add)
            if not bf16_only and write_bf16:
                nc.scalar.copy(xb16[:, :, c0:c0 + nt], xn[:, :, c0:c0 + nt])

        def mm1_tile(r, hc, g8, w1_sb, c0, nt):
            hp = hpp.tile([P, 512], F32, space="PSUM", tag="hp")
            for kg, sh in enumerate((0, -1, 1)):
                for kd in range(DC):
                    kc = kg * DC + kd
                    mmb(hp[:, :nt], w1_sb[:, kc, :],
                        xb16[:, kd, c0 + sh:c0 + sh + nt],
                        kc == 0, kc == KC - 1)
            nc.scalar.activation(g8[:, hc, c0:c0 + nt], hp[:, :nt], AF.Gelu,
                                 bias=b1_sb[:, r, hc:hc + 1])
            nc.vector.tensor_mul(g8[:, hc, c0:c0 + nt],
                                 g8[:, hc, c0:c0 + nt],
                                 rwB[:, r, c0:c0 + nt])

        pre_g8 = None  # next step's r0 g8 (hc0 pre-filled during LN)
        for s in range(S):
            xc, xn = (xA, xB) if s % 2 == 0 else (xB, xA)
            tiles = cfg.step_tiles(s)
            # evolved accumulates in SBUF across rules; weights stream once
            evs = evsp.tile([P, DC, WBUF], F32, tag="evs")
            for r in range(R):
                if r == 0 and pre_g8 is not None:
                    g8, hc_lo = pre_g8, 1
                else:
                    g8 = g8p.tile([P, HC, WBUF], BF16, tag="g8", name="g8")
                    hc_lo = 0
                for hc in range(hc_lo, HC):
                    w1_sb = wp.tile([P, KC, P], BF16, tag="w1")
                    nc.sync.dma_start(out=w1_sb, in_=w1t[r, hc])
                    for (c0, nt) in tiles:
                        mm1_tile(r, hc, g8, w1_sb, c0, nt)
                w2r = w2p.tile([P, HC, DC, P], BF16, tag="w2")
                nc.scalar.dma_start(out=w2r, in_=w2t[r])
                for (c0, nt) in tiles:
                    ev = evp.tile([P, DC, 512], F32, space="PSUM", tag="ev")
                    if r == 0:  # seed: sum_r rw[t,r]*b2[r,d]
                        for dc in range(DC):
                            mmb(ev[:, dc, :nt], b2_sb[:, dc, :],
                                rwT[:, c0:c0 + nt], True, False)
                    for hc in range(HC):
                        for dc in range(DC):
                            mmb(ev[:, dc, :nt], w2r[:, hc, dc, :],
                                g8[:, hc, c0:c0 + nt],
                                r > 0 and hc == 0, hc == HC - 1)
                    if r == 0:
                        nc.vector.tensor_copy(evs[:, :, c0:c0 + nt],
                                              ev[:, :, :nt])
                    else:
                        nc.vector.tensor_add(evs[:, :, c0:c0 + nt],
                                             evs[:, :, c0:c0 + nt],
                                             ev[:, :, :nt])
            if s < S - 1:
                # LN tiles = next step's mm1 read windows (+/-1 col), so each
                # interleaved next-step mm1 tile depends only on the LN tile
                # emitted just before it -> PE bubbles only on the first chain
                nxt = cfg.step_tiles(s + 1)
                pre_g8 = g8p.tile([P, HC, WBUF], BF16, tag="g8")
                w1_sb = wp.tile([P, KC, P], BF16, tag="w1")
                nc.sync.dma_start(out=w1_sb, in_=w1t[0, 0])
                for (c0, nt) in nxt:
                    layer_norm(xc, xn, c0 - 1, nt + 2,
                               evs[:, :, c0 - 1:c0 + nt + 1],
                               ng_sb[:, s, :], nbv_sb[:, s, :])
                    mm1_tile(0, 0, pre_g8, w1_sb, c0, nt)
            else:
                # last CA step: LN (f32 only) interleaved with the final LN
                # (bf16 only) per tile; head consumes xb16 afterwards
                for (c0, nt) in cfg.final_tiles():
                    layer_norm(xc, xn, c0, nt, evs[:, :, c0:c0 + nt],
                               ng_sb[:, s, :], nbv_sb[:, s, :],
                               write_bf16=False)
                    layer_norm(xn, None, c0, nt, None, lg_sb[:, 0, :],
                               lb_sb[:, 0, :], bf16_only=True)

    # ---- head ----------------------------------------------------------
    GW = cfg.VGW * cfg.VPG  # 2000 vocab cols per group
    with tc.tile_pool(name="hwp", bufs=2) as hwp, \
         tc.tile_pool(name="obp", bufs=3) as obp, \
         tc.tile_pool(name="outp", bufs=2, space="PSUM") as outp:
        for vg in range(cfg.NVG):
            hw_sb = hwp.tile([P, DC, GW], BF16, tag="hw")
            nc.sync.dma_start(out=hw_sb, in_=bass.AP(
                headw.tensor, vg * GW, [[cfg.V, P], [P * cfg.V, DC], [1, GW]]))
            for tk in range(cfg.n_tok_chunks):
                c = cfg.own_col0 + tk * P
                po = outp.tile([P, cfg.VPG, 512], F32, space="PSUM", tag="po")
                for dc in range(DC):
                    for vt in range(cfg.VPG):
                        mmb(po[:, vt, :cfg.VGW], xb16[:, dc, c:c + P],
                            hw_sb[:, dc, vt * cfg.VGW:(vt + 1) * cfg.VGW],
                            dc == 0, dc == DC - 1)
                ob = obp.tile([P, cfg.VPG, cfg.VGW], BF16, tag="ob")
                half = cfg.VPG // 2
                nc.vector.tensor_copy(ob[:, :half, :], po[:, :half, :cfg.VGW])
                nc.scalar.copy(ob[:, half:, :], po[:, half:, :cfg.VGW])
                nc.sync.dma_start(
                    out=out[tk * P:(tk + 1) * P, vg * GW:(vg + 1) * GW],
                    in_=ob)


# ---- host-side sharding / unsharding -----------------------------------

def _bf16(a):
    import ml_dtypes
    return np.ascontiguousarray(np.asarray(a, np.float32).astype(
        ml_dtypes.bfloat16))


def shard_inputs(cfg: Cfg, tokens, gate_signal, embed, pos_embed, rule_w1,
                 rule_b1, rule_w2, rule_b2, sel_w, sel_b, norm_g, norm_b,
                 lnf_g, lnf_b, head_w, n_cores=8):
    D, R, T = cfg.D, cfg.R, cfg.T
    w1t = _bf16(np.asarray(rule_w1, np.float32)
                .reshape(R, cfg.KC, P, cfg.HC, P).transpose(0, 3, 2, 1, 4))
    w2t = _bf16(np.asarray(rule_w2, np.float32)
                .reshape(R, cfg.HC, P, cfg.DC, P).transpose(0, 2, 1, 3, 4))
    shared = {
        "embed": np.ascontiguousarray(embed, np.float32),
        "w1t": w1t,
        "b1": np.ascontiguousarray(rule_b1, np.float32),
        "w2t": w2t,
        "b2": _bf16(rule_b2),
        "selw": np.ascontiguousarray(sel_w, np.float32),
        "selb": np.ascontiguousarray(sel_b, np.float32).reshape(1, R),
        "ng": np.ascontiguousarray(norm_g, np.float32),
        "nb": np.ascontiguousarray(norm_b, np.float32),
        "lg": np.ascontiguousarray(lnf_g, np.float32).reshape(1, D),
        "lb": np.ascontiguousarray(lnf_b, np.float32).reshape(1, D),
        "headw": _bf16(head_w),
        "ones1": np.ones((P, 1), np.float32),
        "ones8d": np.ones((8, P), np.float32),
    }
    halves = T // cfg.own
    in_maps = []
    for c in range(n_cores):
        b, h = divmod(c, halves)
        t0 = h * cfg.own
        w = np.arange(t0 - cfg.halo, t0 - cfg.halo + cfg.WBUF) % T
        toks_win = np.asarray(tokens)[b, w].astype(np.int32)
        m = dict(shared)
        m["toksT"] = np.ascontiguousarray(toks_win.reshape(cfg.NT9, P).T)
        m["gate"] = np.ascontiguousarray(
            np.asarray(gate_signal, np.float32)[0, w, :])
        m["pos"] = np.ascontiguousarray(np.asarray(pos_embed, np.float32)[w, :])
        in_maps.append(m)
    return in_maps


def unshard_output(cfg: Cfg, results, n_cores=8):
    halves = cfg.T // cfg.own
    out = np.empty((cfg.B, cfg.T, cfg.V), np.float32)
    for c in range(n_cores):
        b, h = divmod(c, halves)
        out[b, h * cfg.own:(h + 1) * cfg.own, :] = \
            np.asarray(results[c]["out"]).astype(np.float32)
    return out


_NC_CACHE = {}


def kernel(**inputs):
    cfg = Cfg()
    if "full" not in _NC_CACHE:
        _NC_CACHE["full"] = build_nc(cfg)
    nc = _NC_CACHE["full"]
    in_maps = shard_inputs(cfg, **{k: np.asarray(v) for k, v in inputs.items()})
    res = run_bass_kernel_spmd(nc, in_maps, core_ids=list(range(8)))
    return unshard_output(cfg, res.results)


# revision 18
# speedup vs baseline: 1.0079x; 1.0079x over previous
"""Trainium2 Bass kernel for nn_BenchCADecoder (cellular-automaton decoder).

Model: x = embed[tokens]+pos; rw = softmax(gate*1e-3 @ sel_w + sel_b) (step
invariant); 5 CA steps of x = LN(x + sum_r rw[t,r] * MLP_r([x, roll(x,1),
roll(x,-1)])); out = LN_f(x) @ head_w.

Sharding: pure data-parallel over (batch, T-half): 8 cores x 1024 tokens,
each with a circular halo of 5 tokens per side so the 5 neighbor-coupled
steps need zero inter-core communication (window shrinks by 1/side/step).

On-chip layout: x kept transposed [D, tok] so roll() is a free-dim slice
shift and both MLP matmuls run with the contraction on partitions. All MLP
and head matmuls in bf16 (weights pre-cast on host, x cast on-chip once per
step): bf16 enables fast-weight-load so the per-matmul LDWEIGHTS hides
under the previous matmul's stream, and it halves weight DMA. The residual
x stream and all LayerNorm math stay f32. The rule-weighted sum is folded
into PSUM accumulation of the second matmul by pre-scaling gelu outputs
with broadcast rule weights. LayerNorm stats (partition-dim reductions) via
ones-vector matmuls in f32r. Head output is written bf16 and widened to
f32 on host (halves the 131MB/core logit write).
"""

import sys
from contextlib import ExitStack

import numpy as np

sys.path.insert(0, "/opt/trn_rl_repo")

import concourse.bacc as bacc
import concourse.bass as bass
import concourse.mybir as mybir
import concourse.tile as tile
from concourse.bass import IndirectOffsetOnAxis
from concourse.bass_utils import run_bass_kernel_spmd
from concourse.masks import make_identity

F32 = mybir.dt.float32
F32R = mybir.dt.float32r
BF16 = mybir.dt.bfloat16
I32 = mybir.dt.int32
AF = mybir.ActivationFunctionType
OP = mybir.AluOpType

P = 128


class Cfg:
    def __init__(self, D=512, R=8, V=32000, T=2048, B=4, steps=5, own=1024,
                 halo=5, eps=1e-5, gate_scale=1e-3, newton=False):
        self.D, self.R, self.V, self.T, self.B = D, R, V, T, B
        self.steps, self.own, self.halo = steps, own, halo
        self.eps, self.gate_scale = eps, gate_scale
        self.newton = newton
        self.DC = D // P                 # d chunks
        self.HID = 2 * D
        self.HC = self.HID // P          # hidden chunks
        self.KC = 3 * self.DC            # contraction chunks for mm1
        self.WIN = own + 2 * halo        # gathered token window (1034)
        self.NT9 = (self.WIN + P - 1) // P   # gather tiles (9)
        self.WBUF = self.NT9 * P         # x buffer cols (1152)
        self.own_col0 = halo             # first owned col in x buffer
        self.n_tok_chunks = own // P     # head token chunks (8 x 128)
        # head vocab tiling: 16 groups x 4 tiles x 500 cols = 32000
        self.VGW = 500
        self.VPG = 4
        self.NVG = V // (self.VGW * self.VPG)

    def step_tiles(self, s):
        """Output-window tiles for CA step s: [(col0, width)] x3."""
        W = self.own + 2 * (self.steps - 1 - s)
        lo = s + 1
        return self._split3(lo, W)

    def final_tiles(self):
        return self._split3(self.own_col0, self.own)

    @staticmethod
    def _split3(lo, W):
        # three even-width tiles (f32r matmuls require even free sizes)
        w = ((W + 5) // 6) * 2
        return [(lo, w), (lo + w, w), (lo + 2 * w, W - 2 * w)]


def _r(ap):
    """View an f32 AP as float32r for PE consumption."""
    return ap.bitcast(F32R)


def build_nc(cfg: Cfg, num_devices=8):
    """Build the single-core (SPMD) Bass module."""
    nc = bacc.Bacc("TRN2", target_bir_lowering=False, debug=False,
                   num_devices=num_devices)
    D, R, V, HC, KC = cfg.D, cfg.R, cfg.V, cfg.HC, cfg.KC
    S = cfg.steps

    # ---- DRAM I/O ------------------------------------------------------
    toksT = nc.dram_tensor("toksT", [P, cfg.NT9], I32, kind="ExternalInput").ap()
    gate = nc.dram_tensor("gate", [cfg.WBUF, D], F32, kind="ExternalInput").ap()
    pos = nc.dram_tensor("pos", [cfg.WBUF, D], F32, kind="ExternalInput").ap()
    embed = nc.dram_tensor("embed", [V, D], F32, kind="ExternalInput").ap()
    w1t = nc.dram_tensor("w1t", [R, HC, P, KC, P], BF16, kind="ExternalInput").ap()
    b1 = nc.dram_tensor("b1", [R, 2 * D], F32, kind="ExternalInput").ap()
    w2t = nc.dram_tensor("w2t", [R, P, HC, cfg.DC, P], BF16,
                         kind="ExternalInput").ap()
    b2 = nc.dram_tensor("b2", [R, D], BF16, kind="ExternalInput").ap()
    selw = nc.dram_tensor("selw", [D, R], F32R, kind="ExternalInput").ap()
    selb = nc.dram_tensor("selb", [1, R], F32R, kind="ExternalInput").ap()
    ng = nc.dram_tensor("ng", [S, D], F32, kind="ExternalInput").ap()
    nb_ = nc.dram_tensor("nb", [S, D], F32, kind="ExternalInput").ap()
    lg = nc.dram_tensor("lg", [1, D], F32, kind="ExternalInput").ap()
    lb = nc.dram_tensor("lb", [1, D], F32, kind="ExternalInput").ap()
    headw = nc.dram_tensor("headw", [D, V], BF16, kind="ExternalInput").ap()
    ones1_d = nc.dram_tensor("ones1", [P, 1], F32R, kind="ExternalInput").ap()
    ones8_d = nc.dram_tensor("ones8d", [8, P], F32R, kind="ExternalInput").ap()
    out = nc.dram_tensor("out", [cfg.own, V], BF16, kind="ExternalOutput").ap()

    with ExitStack() as ctx:
        ctx.enter_context(nc.allow_low_precision(reason="bf16 mms by design"))
        tc = ctx.enter_context(tile.TileContext(nc))
        _emit(ctx, tc, cfg, toksT, gate, pos, embed, w1t, b1, w2t, b2, selw,
              selb, ng, nb_, lg, lb, headw, out, ones1_d, ones8_d)
    nc.compile()
    return nc


def _emit(ctx, tc, cfg, toksT, gate, pos, embed, w1t, b1, w2t, b2, selw,
          selb, ng, nb_, lg, lb, headw, out, ones1_d, ones8_d):
    nc = tc.nc
    D, R, DC, HC, KC = cfg.D, cfg.R, cfg.DC, cfg.HC, cfg.KC
    S, WBUF, NT9 = cfg.steps, cfg.WBUF, cfg.NT9

    def mmr(o, lh, rh, start, stop):
        nc.tensor.matmul(o, _r(lh), _r(rh), start=start, stop=stop)

    def mmb(o, lh, rh, start, stop):
        nc.tensor.matmul(o, lh, rh, start=start, stop=stop)

    # ---- persistent SBUF ----------------------------------------------
    persist = ctx.enter_context(tc.tile_pool(name="persist", bufs=1))
    xA = persist.tile([P, DC, WBUF], F32R, name="xA")
    xB = persist.tile([P, DC, WBUF], F32R, name="xB")
    xb16 = persist.tile([P, DC, WBUF], BF16, name="xb16")
    rbF = persist.tile([P, DC, WBUF], F32R, name="rbF")   # x + evolved
    sqB = persist.tile([P, DC, WBUF], BF16, name="sqB")   # (x + evolved)^2
    ones128b = persist.tile([P, 1], BF16, name="ones128b")
    epsT = persist.tile([P, 1], F32, name="epsT")
    rwB = persist.tile([P, R, WBUF], BF16, name="rwB")   # bcast rule weights
    rwT = persist.tile([R, WBUF], BF16, name="rwT")      # rw [r, tok]
    ident = persist.tile([P, P], F32, name="ident")
    ones128 = persist.tile([P, 1], F32R, name="ones128")
    ones8 = persist.tile([8, P], F32R, name="ones8")
    b1_sb = persist.tile([P, R, HC], F32, name="b1_sb")
    b2_sb = persist.tile([R, DC, P], BF16, name="b2_sb")
    ng_sb = persist.tile([P, S, DC], F32, name="ng_sb")
    nbv_sb = persist.tile([P, S, DC], F32, name="nbv_sb")
    lg_sb = persist.tile([P, 1, DC], F32, name="lg_sb")
    lb_sb = persist.tile([P, 1, DC], F32, name="lb_sb")

    make_identity(nc, ident)
    nc.sync.dma_start(out=ones128, in_=ones1_d)
    nc.sync.dma_start(out=ones8[0:8, :], in_=ones8_d)
    nc.vector.memset(ones128b, 1.0)
    nc.vector.memset(epsT, cfg.eps)

    nc.sync.dma_start(out=b1_sb, in_=bass.AP(
        b1.tensor, 0, [[1, P], [2 * D, R], [P, HC]]))
    nc.sync.dma_start(out=b2_sb, in_=bass.AP(
        b2.tensor, 0, [[D, R], [P, DC], [1, P]]))
    nc.sync.dma_start(out=ng_sb, in_=bass.AP(
        ng.tensor, 0, [[1, P], [D, S], [P, DC]]))
    nc.sync.dma_start(out=nbv_sb, in_=bass.AP(
        nb_.tensor, 0, [[1, P], [D, S], [P, DC]]))
    nc.sync.dma_start(out=lg_sb, in_=bass.AP(
        lg.tensor, 0, [[1, P], [D, 1], [P, DC]]))
    nc.sync.dma_start(out=lb_sb, in_=bass.AP(
        lb.tensor, 0, [[1, P], [D, 1], [P, DC]]))

    # ---- setup: embed gather + pos -> xA/xb16; gate -> rw --------------
    with tc.tile_pool(name="setup", bufs=3) as sp, \
         tc.tile_pool(name="setup_ps", bufs=2, space="PSUM") as spp, \
         tc.tile_pool(name="setup_small", bufs=2) as ss:
        idx = persist.tile([P, NT9], I32, name="idx")
        nc.sync.dma_start(out=idx, in_=toksT)
        selw_sb = persist.tile([P, DC, R], F32R, name="selw_sb")
        nc.sync.dma_start(out=selw_sb, in_=bass.AP(
            selw.tensor, 0, [[R, P], [P * R, DC], [1, R]]))
        nc.vector.tensor_scalar_mul(selw_sb, selw_sb, cfg.gate_scale)
        selb_sb = persist.tile([1, R], F32R, name="selb_sb")
        nc.sync.dma_start(out=selb_sb, in_=selb)

        for i in range(NT9):
            # x tile: gather embed rows + pos
            xg = sp.tile([P, D], F32, tag="xg")
            nc.gpsimd.indirect_dma_start(
                out=xg, out_offset=None, in_=embed,
                in_offset=IndirectOffsetOnAxis(ap=idx[:, i:i + 1], axis=0))
            pt = sp.tile([P, D], F32, tag="pt")
            nc.sync.dma_start(out=pt, in_=pos[i * P:(i + 1) * P, :])
            nc.vector.tensor_add(xg, xg, pt)
            tp = spp.tile([P, DC, P], F32, space="PSUM", tag="tp")
            for dc in range(DC):
                nc.tensor.transpose(tp[:, dc, :], xg[:, dc * P:(dc + 1) * P], ident)
            nc.vector.tensor_copy(xA[:, :, i * P:(i + 1) * P], tp)
            nc.scalar.copy(xb16[:, :, i * P:(i + 1) * P], tp)

            # gate tile -> gateT (transposed), then logits -> rw
            gt = sp.tile([P, D], F32, tag="gt")
            nc.sync.dma_start(out=gt, in_=gate[i * P:(i + 1) * P, :])
            tg = spp.tile([P, DC, P], F32, space="PSUM", tag="tp")
            for dc in range(DC):
                nc.tensor.transpose(tg[:, dc, :], gt[:, dc * P:(dc + 1) * P], ident)
            gT = sp.tile([P, DC, P], F32R, tag="gT")
            nc.vector.tensor_copy(gT, tg)

            lp = spp.tile([P, R], F32, space="PSUM", tag="lp")
            for dc in range(DC):
                mmr(lp, gT[:, dc, :], selw_sb[:, dc, :], dc == 0, False)
            mmr(lp, ones8[0:1, :], selb_sb, False, True)  # rank-1 +sel_b
            e = ss.tile([P, R], F32, tag="e")
            nc.scalar.activation(e, lp, AF.Exp)
            esum = ss.tile([P, 1], F32, tag="es")
            nc.vector.tensor_reduce(esum, e, mybir.AxisListType.X, OP.add)
            nc.vector.reciprocal(esum, esum)
            nc.vector.tensor_scalar(out=e, in0=e, scalar1=esum, scalar2=None,
                                    op0=OP.mult)
            rp = spp.tile([R, P], F32, space="PSUM", tag="rp")
            nc.tensor.transpose(rp, e, ident)
            nc.vector.tensor_copy(rwT[:, i * P:(i + 1) * P], rp)

        # broadcast rw rows across partitions: rwB[p, r, c] = rw[tok c, r]
        # (bounce via DRAM: SBUF sources cannot have partition step 0)
        rw_dram = nc.dram_tensor("rw_scratch", [R, WBUF], BF16).ap()
        nc.sync.dma_start(out=rw_dram, in_=rwT)
        for r in range(R):
            nc.sync.dma_start(
                out=rwB[:, r, :],
                in_=bass.AP(rw_dram.tensor, r * WBUF, [[0, P], [1, WBUF]]))

    # ---- CA steps ------------------------------------------------------
    with tc.tile_pool(name="w1p", bufs=3) as wp, \
         tc.tile_pool(name="w2p", bufs=2) as w2p, \
         tc.tile_pool(name="g8p", bufs=2) as g8p, \
         tc.tile_pool(name="evsp", bufs=1) as evsp, \
         tc.tile_pool(name="rbp", bufs=1) as rp_, \
         tc.tile_pool(name="rowp", bufs=1) as rowp, \
         tc.tile_pool(name="evp", bufs=1, space="PSUM") as evp, \
         tc.tile_pool(name="hpp", bufs=2, space="PSUM") as hpp, \
         tc.tile_pool(name="stp", bufs=1, space="PSUM") as stp:

        def ln_core(rb_src, c0, nt, g_col, b_col, xn, bf16_only=False,
                    write_bf16=True):
            """LN over cols [c0, c0+nt) from rb_src (+ sqB) -> xn / xb16.

            rb_src and sqB[:, :, c0:c0+nt] must already hold x+ev and its
            square (computed in the mm2 shadow).
            """
            inv_d = 1.0 / D
            rb = rb_src[:, :, c0:c0 + nt]
            st_s = stp.tile([1, 512], F32, space="PSUM", tag="sts")
            st_q = stp.tile([1, 512], F32, space="PSUM", tag="stq")
            for dc in range(DC):
                nc.tensor.matmul(st_s[:, :nt], ones128, rb[:, dc, :],
                                 start=dc == 0, stop=dc == DC - 1)
            for dc in range(DC):
                nc.tensor.matmul(st_q[:, :nt], ones128b,
                                 sqB[:, dc, c0:c0 + nt],
                                 start=dc == 0, stop=dc == DC - 1)
            mrow = rowp.tile([1, nt], F32, tag="mrow")
            nc.vector.tensor_scalar_mul(mrow, st_s[:, :nt], inv_d)
            msq = rowp.tile([1, nt], F32, tag="msq")
            nc.vector.tensor_mul(msq, mrow, mrow)
            wrow = rowp.tile([1, nt], F32, tag="wrow")
            # wrow = st_q/D - m^2
            nc.vector.scalar_tensor_tensor(out=wrow, in0=st_q[:, :nt],
                                           scalar=inv_d, in1=msq,
                                           op0=OP.mult, op1=OP.subtract)
            srow = rowp.tile([1, nt], F32R, tag="srow")
            nc.scalar.activation(srow, wrow, AF.Sqrt, bias=epsT[0:1, :])
            nc.vector.reciprocal(srow, srow)
            # nms = -m*s
            nms = rowp.tile([1, nt], F32R, tag="nms")
            nc.vector.scalar_tensor_tensor(out=nms, in0=mrow, scalar=-1.0,
                                           in1=srow, op0=OP.mult, op1=OP.mult)
            bc = evp.tile([P, 2, 512], F32, space="PSUM", tag="ev")
            nc.tensor.matmul(bc[:, 0, :nt], ones8[0:1, :], srow,
                             start=True, stop=True)
            nc.tensor.matmul(bc[:, 1, :nt], ones8[0:1, :], nms,
                             start=True, stop=True)
            u = rp_.tile([P, DC, nt], F32, tag="u")
            nc.vector.tensor_mul(u, rb,
                                 bc[:, 0:1, :nt].broadcast_to([P, DC, nt]))
            nc.vector.tensor_add(u, u,
                                 bc[:, 1:2, :nt].broadcast_to([P, DC, nt]))
            for dc in range(DC):
                dst = xb16 if bf16_only else xn
                nc.scalar.activation(
                    out=dst[:, dc, c0:c0 + nt], in_=u[:, dc, :],
                    func=AF.Identity,
                    bias=b_col[:, dc:dc + 1], scale=g_col[:, dc:dc + 1])
            if not bf16_only and write_bf16:
                nc.scalar.copy(xb16[:, :, c0:c0 + nt], xn[:, :, c0:c0 + nt])

        def mm1_tile(r, hc, g8, w1_sb, c0, nt):
            hp = hpp.tile([P, 512], F32, space="PSUM", tag="hp")
            for kg, sh in enumerate((0, -1, 1)):
                for kd in range(DC):
                    kc = kg * DC + kd
                    mmb(hp[:, :nt], w1_sb[:, kc, :],
                        xb16[:, kd, c0 + sh:c0 + sh + nt],
                        kc == 0, kc == KC - 1)
            nc.scalar.activation(g8[:, hc, c0:c0 + nt], hp[:, :nt], AF.Gelu,
                                 bias=b1_sb[:, r, hc:hc + 1])
            nc.vector.tensor_mul(g8[:, hc, c0:c0 + nt],
                                 g8[:, hc, c0:c0 + nt],
                                 rwB[:, r, c0:c0 + nt])

        pre_g8 = None  # next step's r0 g8 (hc0 pre-filled during LN)
        for s in range(S):
            xc, xn = (xA, xB) if s % 2 == 0 else (xB, xA)
            tiles = cfg.step_tiles(s)
            # evolved accumulates in SBUF across rules; weights stream once
            evs = evsp.tile([P, DC, WBUF], F32, tag="evs")
            for r in range(R):
                if r == 0 and pre_g8 is not None:
                    g8, hc_lo = pre_g8, 1
                else:
                    g8 = g8p.tile([P, HC, WBUF], BF16, tag="g8", name="g8")
                    hc_lo = 0
                for hc in range(hc_lo, HC):
                    w1_sb = wp.tile([P, KC, P], BF16, tag="w1")
                    nc.sync.dma_start(out=w1_sb, in_=w1t[r, hc])
                    for (c0, nt) in tiles:
                        mm1_tile(r, hc, g8, w1_sb, c0, nt)
                w2r = w2p.tile([P, HC, DC, P], BF16, tag="w2")
                nc.scalar.dma_start(out=w2r, in_=w2t[r])
                last = r == R - 1
                for (c0, nt) in tiles:
                    ev = evp.tile([P, DC, 512], F32, space="PSUM", tag="ev")
                    if r == 0:  # seed: sum_r rw[t,r]*b2[r,d]
                        for dc in range(DC):
                            mmb(ev[:, dc, :nt], b2_sb[:, dc, :],
                                rwT[:, c0:c0 + nt], True, False)
                    for hc in range(HC):
                        for dc in range(DC):
                            mmb(ev[:, dc, :nt], w2r[:, hc, dc, :],
                                g8[:, hc, c0:c0 + nt],
                                r > 0 and hc == 0, hc == HC - 1)
                    if r == 0:
                        nc.vector.tensor_copy(evs[:, :, c0:c0 + nt],
                                              ev[:, :, :nt])
                    else:
                        nc.vector.tensor_add(evs[:, :, c0:c0 + nt],
                                             evs[:, :, c0:c0 + nt],
                                             ev[:, :, :nt])
                    if last:
                        # rb = x + evolved and its square, in the mm2 shadow
                        nc.vector.tensor_add(rbF[:, :, c0:c0 + nt],
                                             xc[:, :, c0:c0 + nt],
                                             evs[:, :, c0:c0 + nt])
                        nc.scalar.square(sqB[:, :, c0:c0 + nt],
                                         rbF[:, :, c0:c0 + nt])
            if s < S - 1:
                # LN tiles = next step's mm1 read windows (+/-1 col), so each
                # interleaved next-step mm1 tile depends only on the LN tile
                # emitted just before it
                nxt = cfg.step_tiles(s + 1)
                pre_g8 = g8p.tile([P, HC, WBUF], BF16, tag="g8")
                w1_sb = wp.tile([P, KC, P], BF16, tag="w1")
                nc.sync.dma_start(out=w1_sb, in_=w1t[0, 0])
                for (c0, nt) in nxt:
                    ln_core(rbF, c0 - 1, nt + 2, ng_sb[:, s, :],
                            nbv_sb[:, s, :], xn)
                    mm1_tile(0, 0, pre_g8, w1_sb, c0, nt)
            else:
                # last CA step LN (f32 only) + final LN (bf16 only) per tile;
                # head consumes xb16 afterwards
                fins = cfg.final_tiles()
                done4 = []
                for j, (c0, nt) in enumerate(fins):
                    ln_core(rbF, c0, nt, ng_sb[:, s, :], nbv_sb[:, s, :],
                            xn, write_bf16=False)
                    nc.scalar.square(sqB[:, :, c0:c0 + nt],
                                     xn[:, :, c0:c0 + nt])
                    done4.append((c0, nt))
                    if j >= 1:
                        (fc0, fnt) = done4.pop(0)
                        ln_core(xn, fc0, fnt, lg_sb[:, 0, :], lb_sb[:, 0, :],
                                None, bf16_only=True)
                for (fc0, fnt) in done4:
                    ln_core(xn, fc0, fnt, lg_sb[:, 0, :], lb_sb[:, 0, :],
                            None, bf16_only=True)

    # ---- head ----------------------------------------------------------
    GW = cfg.VGW * cfg.VPG  # 2000 vocab cols per group
    with tc.tile_pool(name="hwp", bufs=2) as hwp, \
         tc.tile_pool(name="obp", bufs=3) as obp, \
         tc.tile_pool(name="outp", bufs=2, space="PSUM") as outp:
        for vg in range(cfg.NVG):
            hw_sb = hwp.tile([P, DC, GW], BF16, tag="hw")
            nc.sync.dma_start(out=hw_sb, in_=bass.AP(
                headw.tensor, vg * GW, [[cfg.V, P], [P * cfg.V, DC], [1, GW]]))
            for tk in range(cfg.n_tok_chunks):
                c = cfg.own_col0 + tk * P
                po = outp.tile([P, cfg.VPG, 512], F32, space="PSUM", tag="po")
                for dc in range(DC):
                    for vt in range(cfg.VPG):
                        mmb(po[:, vt, :cfg.VGW], xb16[:, dc, c:c + P],
                            hw_sb[:, dc, vt * cfg.VGW:(vt + 1) * cfg.VGW],
                            dc == 0, dc == DC - 1)
                ob = obp.tile([P, cfg.VPG, cfg.VGW], BF16, tag="ob")
                half = cfg.VPG // 2
                nc.vector.tensor_copy(ob[:, :half, :], po[:, :half, :cfg.VGW])
                nc.scalar.copy(ob[:, half:, :], po[:, half:, :cfg.VGW])
                nc.sync.dma_start(
                    out=out[tk * P:(tk + 1) * P, vg * GW:(vg + 1) * GW],
                    in_=ob)


# ---- host-side sharding / unsharding -----------------------------------

def _bf16(a):
    import ml_dtypes
    return np.ascontiguousarray(np.asarray(a, np.float32).astype(
        ml_dtypes.bfloat16))


def shard_inputs(cfg: Cfg, tokens, gate_signal, embed, pos_embed, rule_w1,
                 rule_b1, rule_w2, rule_b2, sel_w, sel_b, norm_g, norm_b,
                 lnf_g, lnf_b, head_w, n_cores=8):
    D, R, T = cfg.D, cfg.R, cfg.T
    w1t = _bf16(np.asarray(rule_w1, np.float32)
                .reshape(R, cfg.KC, P, cfg.HC, P).transpose(0, 3, 2, 1, 4))
    w2t = _bf16(np.asarray(rule_w2, np.float32)
                .reshape(R, cfg.HC, P, cfg.DC, P).transpose(0, 2, 1, 3, 4))
    shared = {
        "embed": np.ascontiguousarray(embed, np.float32),
        "w1t": w1t,
        "b1": np.ascontiguousarray(rule_b1, np.float32),
        "w2t": w2t,
        "b2": _bf16(rule_b2),
        "selw": np.ascontiguousarray(sel_w, np.float32),
        "selb": np.ascontiguousarray(sel_b, np.float32).reshape(1, R),
        "ng": np.ascontiguousarray(norm_g, np.float32),
        "nb": np.ascontiguousarray(norm_b, np.float32),
        "lg": np.ascontiguousarray(lnf_g, np.float32).reshape(1, D),
        "lb": np.ascontiguousarray(lnf_b, np.float32).reshape(1, D),
        "headw": _bf16(head_w),
        "ones1": np.ones((P, 1), np.float32),
        "ones8d": np.ones((8, P), np.float32),
    }
    halves = T // cfg.own
    in_maps = []
    for c in range(n_cores):
        b, h = divmod(c, halves)
        t0 = h * cfg.own
        w = np.arange(t0 - cfg.halo, t0 - cfg.halo + cfg.WBUF) % T
        toks_win = np.asarray(tokens)[b, w].astype(np.int32)
        m = dict(shared)
        m["toksT"] = np.ascontiguousarray(toks_win.reshape(cfg.NT9, P).T)
        m["gate"] = np.ascontiguousarray(
            np.asarray(gate_signal, np.float32)[0, w, :])
        m["pos"] = np.ascontiguousarray(np.asarray(pos_embed, np.float32)[w, :])
        in_maps.append(m)
    return in_maps


def unshard_output(cfg: Cfg, results, n_cores=8):
    halves = cfg.T // cfg.own
    out = np.empty((cfg.B, cfg.T, cfg.V), np.float32)
    for c in range(n_cores):
        b, h = divmod(c, halves)
        out[b, h * cfg.own:(h + 1) * cfg.own, :] = \
            np.asarray(results[c]["out"]).astype(np.float32)
    return out


_NC_CACHE = {}


def kernel(**inputs):
    cfg = Cfg()
    if "full" not in _NC_CACHE:
        _NC_CACHE["full"] = build_nc(cfg)
    nc = _NC_CACHE["full"]
    in_maps = shard_inputs(cfg, **{k: np.asarray(v) for k, v in inputs.items()})
    res = run_bass_kernel_spmd(nc, in_maps, core_ids=list(range(8)))
    return unshard_output(cfg, res.results)


# revision 22
# speedup vs baseline: 1.0462x; 1.0379x over previous
"""Trainium2 Bass kernel for nn_BenchCADecoder (cellular-automaton decoder).

Model: x = embed[tokens]+pos; rw = softmax(gate*1e-3 @ sel_w + sel_b) (step
invariant); 5 CA steps of x = LN(x + sum_r rw[t,r] * MLP_r([x, roll(x,1),
roll(x,-1)])); out = LN_f(x) @ head_w.

Sharding: pure data-parallel over (batch, T-half): 8 cores x 1024 tokens,
each with a circular halo of 5 tokens per side so the 5 neighbor-coupled
steps need zero inter-core communication (window shrinks by 1/side/step).

On-chip layout: x kept transposed [D, tok] so roll() is a free-dim slice
shift and both MLP matmuls run with the contraction on partitions. All MLP
and head matmuls in bf16 (weights pre-cast on host, x cast on-chip once per
step): bf16 enables fast-weight-load so the per-matmul LDWEIGHTS hides
under the previous matmul's stream, and it halves weight DMA. The residual
x stream and all LayerNorm math stay f32. The rule-weighted sum is folded
into PSUM accumulation of the second matmul by pre-scaling gelu outputs
with broadcast rule weights. LayerNorm stats (partition-dim reductions) via
ones-vector matmuls in f32r. Head output is written bf16 and widened to
f32 on host (halves the 131MB/core logit write).
"""

import sys
from contextlib import ExitStack

import numpy as np

sys.path.insert(0, "/opt/trn_rl_repo")

import concourse.bacc as bacc
import concourse.bass as bass
import concourse.mybir as mybir
import concourse.tile as tile
from concourse.bass import IndirectOffsetOnAxis
from concourse.bass_utils import run_bass_kernel_spmd
from concourse.masks import make_identity

F32 = mybir.dt.float32
F32R = mybir.dt.float32r
BF16 = mybir.dt.bfloat16
I32 = mybir.dt.int32
AF = mybir.ActivationFunctionType
OP = mybir.AluOpType

P = 128


class Cfg:
    def __init__(self, D=512, R=8, V=32000, T=2048, B=4, steps=5, own=1024,
                 halo=5, eps=1e-5, gate_scale=1e-3, newton=False):
        self.D, self.R, self.V, self.T, self.B = D, R, V, T, B
        self.steps, self.own, self.halo = steps, own, halo
        self.eps, self.gate_scale = eps, gate_scale
        self.newton = newton
        self.DC = D // P                 # d chunks
        self.HID = 2 * D
        self.HC = self.HID // P          # hidden chunks
        self.KC = 3 * self.DC            # contraction chunks for mm1
        self.WIN = own + 2 * halo        # gathered token window (1034)
        self.NT9 = (self.WIN + P - 1) // P   # gather tiles (9)
        self.WBUF = self.NT9 * P         # x buffer cols (1152)
        self.own_col0 = halo             # first owned col in x buffer
        self.n_tok_chunks = own // P     # head token chunks (8 x 128)
        # head vocab tiling: 16 groups x 4 tiles x 500 cols = 32000
        self.VGW = 500
        self.VPG = 4
        self.NVG = V // (self.VGW * self.VPG)

    def step_tiles(self, s):
        """Output-window tiles for CA step s: [(col0, width)] x3."""
        W = self.own + 2 * (self.steps - 1 - s)
        lo = s + 1
        return self._split3(lo, W)

    def final_tiles(self):
        return self._split3(self.own_col0, self.own)

    @staticmethod
    def _split3(lo, W):
        # three even-width tiles (f32r matmuls require even free sizes)
        w = ((W + 5) // 6) * 2
        return [(lo, w), (lo + w, w), (lo + 2 * w, W - 2 * w)]


def _r(ap):
    """View an f32 AP as float32r for PE consumption."""
    return ap.bitcast(F32R)


def build_nc(cfg: Cfg, num_devices=8):
    """Build the single-core (SPMD) Bass module."""
    nc = bacc.Bacc("TRN2", target_bir_lowering=False, debug=False,
                   num_devices=num_devices)
    D, R, V, HC, KC = cfg.D, cfg.R, cfg.V, cfg.HC, cfg.KC
    S = cfg.steps

    # ---- DRAM I/O ------------------------------------------------------
    toksT = nc.dram_tensor("toksT", [P, cfg.NT9], I32, kind="ExternalInput").ap()
    gate = nc.dram_tensor("gate", [cfg.WBUF, D], F32, kind="ExternalInput").ap()
    pos = nc.dram_tensor("pos", [cfg.WBUF, D], F32, kind="ExternalInput").ap()
    embed = nc.dram_tensor("embed", [V, D], F32, kind="ExternalInput").ap()
    w1t = nc.dram_tensor("w1t", [R, HC, P, KC, P], BF16, kind="ExternalInput").ap()
    b1 = nc.dram_tensor("b1", [R, 2 * D], F32, kind="ExternalInput").ap()
    w2t = nc.dram_tensor("w2t", [R, P, HC, cfg.DC, P], BF16,
                         kind="ExternalInput").ap()
    b2 = nc.dram_tensor("b2", [R, D], BF16, kind="ExternalInput").ap()
    selw = nc.dram_tensor("selw", [D, R], F32R, kind="ExternalInput").ap()
    selb = nc.dram_tensor("selb", [1, R], F32R, kind="ExternalInput").ap()
    ng = nc.dram_tensor("ng", [S, D], F32, kind="ExternalInput").ap()
    nb_ = nc.dram_tensor("nb", [S, D], F32, kind="ExternalInput").ap()
    lg = nc.dram_tensor("lg", [1, D], F32, kind="ExternalInput").ap()
    lb = nc.dram_tensor("lb", [1, D], F32, kind="ExternalInput").ap()
    headw = nc.dram_tensor("headw", [D, V], BF16, kind="ExternalInput").ap()
    ones1_d = nc.dram_tensor("ones1", [P, 1], F32R, kind="ExternalInput").ap()
    ones8_d = nc.dram_tensor("ones8d", [8, P], F32R, kind="ExternalInput").ap()
    out = nc.dram_tensor("out", [cfg.own, V], BF16, kind="ExternalOutput").ap()

    with ExitStack() as ctx:
        ctx.enter_context(nc.allow_low_precision(reason="bf16 mms by design"))
        tc = ctx.enter_context(tile.TileContext(nc))
        _emit(ctx, tc, cfg, toksT, gate, pos, embed, w1t, b1, w2t, b2, selw,
              selb, ng, nb_, lg, lb, headw, out, ones1_d, ones8_d)
    nc.compile()
    return nc


def _emit(ctx, tc, cfg, toksT, gate, pos, embed, w1t, b1, w2t, b2, selw,
          selb, ng, nb_, lg, lb, headw, out, ones1_d, ones8_d):
    nc = tc.nc
    D, R, DC, HC, KC = cfg.D, cfg.R, cfg.DC, cfg.HC, cfg.KC
    S, WBUF, NT9 = cfg.steps, cfg.WBUF, cfg.NT9

    def mmr(o, lh, rh, start, stop):
        nc.tensor.matmul(o, _r(lh), _r(rh), start=start, stop=stop)

    def mmb(o, lh, rh, start, stop):
        nc.tensor.matmul(o, lh, rh, start=start, stop=stop)

    # ---- persistent SBUF ----------------------------------------------
    persist = ctx.enter_context(tc.tile_pool(name="persist", bufs=1))
    xA = persist.tile([P, DC, WBUF], F32R, name="xA")
    xB = persist.tile([P, DC, WBUF], F32R, name="xB")
    xb16 = persist.tile([P, DC, WBUF], BF16, name="xb16")
    rbF = persist.tile([P, DC, WBUF], F32R, name="rbF")   # x + evolved
    sqB = persist.tile([P, DC, WBUF], BF16, name="sqB")   # (x + evolved)^2
    ones128b = persist.tile([P, 1], BF16, name="ones128b")
    epsT = persist.tile([P, 1], F32, name="epsT")
    rwB = persist.tile([P, R, WBUF], BF16, name="rwB")   # bcast rule weights
    rwT = persist.tile([R, WBUF], BF16, name="rwT")      # rw [r, tok]
    ident = persist.tile([P, P], F32, name="ident")
    ones128 = persist.tile([P, 1], F32R, name="ones128")
    ones8 = persist.tile([8, P], F32R, name="ones8")
    b1_sb = persist.tile([P, R, HC], F32, name="b1_sb")
    b2_sb = persist.tile([R, DC, P], BF16, name="b2_sb")
    ng_sb = persist.tile([P, S, DC], F32, name="ng_sb")
    nbv_sb = persist.tile([P, S, DC], F32, name="nbv_sb")
    lg_sb = persist.tile([P, 1, DC], F32, name="lg_sb")
    lb_sb = persist.tile([P, 1, DC], F32, name="lb_sb")

    make_identity(nc, ident)
    nc.sync.dma_start(out=ones128, in_=ones1_d)
    nc.sync.dma_start(out=ones8[0:8, :], in_=ones8_d)
    nc.vector.memset(ones128b, 1.0)
    nc.vector.memset(epsT, cfg.eps)

    nc.sync.dma_start(out=b1_sb, in_=bass.AP(
        b1.tensor, 0, [[1, P], [2 * D, R], [P, HC]]))
    nc.sync.dma_start(out=b2_sb, in_=bass.AP(
        b2.tensor, 0, [[D, R], [P, DC], [1, P]]))
    nc.sync.dma_start(out=ng_sb, in_=bass.AP(
        ng.tensor, 0, [[1, P], [D, S], [P, DC]]))
    nc.sync.dma_start(out=nbv_sb, in_=bass.AP(
        nb_.tensor, 0, [[1, P], [D, S], [P, DC]]))
    nc.sync.dma_start(out=lg_sb, in_=bass.AP(
        lg.tensor, 0, [[1, P], [D, 1], [P, DC]]))
    nc.sync.dma_start(out=lb_sb, in_=bass.AP(
        lb.tensor, 0, [[1, P], [D, 1], [P, DC]]))

    # ---- setup: embed gather + pos -> xA/xb16; gate -> rw --------------
    with tc.tile_pool(name="setup", bufs=3) as sp, \
         tc.tile_pool(name="setup_ps", bufs=2, space="PSUM") as spp, \
         tc.tile_pool(name="setup_small", bufs=2) as ss:
        idx = persist.tile([P, NT9], I32, name="idx")
        nc.sync.dma_start(out=idx, in_=toksT)
        selw_sb = persist.tile([P, DC, R], F32R, name="selw_sb")
        nc.sync.dma_start(out=selw_sb, in_=bass.AP(
            selw.tensor, 0, [[R, P], [P * R, DC], [1, R]]))
        nc.vector.tensor_scalar_mul(selw_sb, selw_sb, cfg.gate_scale)
        selb_sb = persist.tile([1, R], F32R, name="selb_sb")
        nc.sync.dma_start(out=selb_sb, in_=selb)

        for i in range(NT9):
            # x tile: gather embed rows + pos
            xg = sp.tile([P, D], F32, tag="xg")
            nc.gpsimd.indirect_dma_start(
                out=xg, out_offset=None, in_=embed,
                in_offset=IndirectOffsetOnAxis(ap=idx[:, i:i + 1], axis=0))
            pt = sp.tile([P, D], F32, tag="pt")
            nc.sync.dma_start(out=pt, in_=pos[i * P:(i + 1) * P, :])
            nc.vector.tensor_add(xg, xg, pt)
            tp = spp.tile([P, DC, P], F32, space="PSUM", tag="tp")
            for dc in range(DC):
                nc.tensor.transpose(tp[:, dc, :], xg[:, dc * P:(dc + 1) * P], ident)
            nc.vector.tensor_copy(xA[:, :, i * P:(i + 1) * P], tp)
            nc.scalar.copy(xb16[:, :, i * P:(i + 1) * P], tp)

            # gate tile -> gateT (transposed), then logits -> rw
            gt = sp.tile([P, D], F32, tag="gt")
            nc.sync.dma_start(out=gt, in_=gate[i * P:(i + 1) * P, :])
            tg = spp.tile([P, DC, P], F32, space="PSUM", tag="tp")
            for dc in range(DC):
                nc.tensor.transpose(tg[:, dc, :], gt[:, dc * P:(dc + 1) * P], ident)
            gT = sp.tile([P, DC, P], F32R, tag="gT")
            nc.vector.tensor_copy(gT, tg)

            lp = spp.tile([P, R], F32, space="PSUM", tag="lp")
            for dc in range(DC):
                mmr(lp, gT[:, dc, :], selw_sb[:, dc, :], dc == 0, False)
            mmr(lp, ones8[0:1, :], selb_sb, False, True)  # rank-1 +sel_b
            e = ss.tile([P, R], F32, tag="e")
            nc.scalar.activation(e, lp, AF.Exp)
            esum = ss.tile([P, 1], F32, tag="es")
            nc.vector.tensor_reduce(esum, e, mybir.AxisListType.X, OP.add)
            nc.vector.reciprocal(esum, esum)
            nc.vector.tensor_scalar(out=e, in0=e, scalar1=esum, scalar2=None,
                                    op0=OP.mult)
            rp = spp.tile([R, P], F32, space="PSUM", tag="rp")
            nc.tensor.transpose(rp, e, ident)
            nc.vector.tensor_copy(rwT[:, i * P:(i + 1) * P], rp)

        # broadcast rw rows across partitions: rwB[p, r, c] = rw[tok c, r]
        # (bounce via DRAM: SBUF sources cannot have partition step 0)
        rw_dram = nc.dram_tensor("rw_scratch", [R, WBUF], BF16).ap()
        nc.sync.dma_start(out=rw_dram, in_=rwT)
        for r in range(R):
            nc.sync.dma_start(
                out=rwB[:, r, :],
                in_=bass.AP(rw_dram.tensor, r * WBUF, [[0, P], [1, WBUF]]))

    # ---- CA steps ------------------------------------------------------
    with tc.tile_pool(name="w1p", bufs=3) as wp, \
         tc.tile_pool(name="w2p", bufs=2) as w2p, \
         tc.tile_pool(name="g8p", bufs=2) as g8p, \
         tc.tile_pool(name="evsp", bufs=1) as evsp, \
         tc.tile_pool(name="rbp", bufs=1) as rp_, \
         tc.tile_pool(name="rowp", bufs=1) as rowp, \
         tc.tile_pool(name="evp", bufs=1, space="PSUM") as evp, \
         tc.tile_pool(name="hpp", bufs=2, space="PSUM") as hpp, \
         tc.tile_pool(name="stp", bufs=1, space="PSUM") as stp:

        def ln_core(rb_src, c0, nt, g_col, b_col, xn, bf16_only=False,
                    write_bf16=True):
            """LN over cols [c0, c0+nt) from rb_src (+ sqB) -> xn / xb16.

            rb_src and sqB[:, :, c0:c0+nt] must already hold x+ev and its
            square (computed in the mm2 shadow).
            """
            inv_d = 1.0 / D
            rb = rb_src[:, :, c0:c0 + nt]
            st_s = stp.tile([1, 512], F32, space="PSUM", tag="sts")
            st_q = stp.tile([1, 512], F32, space="PSUM", tag="stq")
            for dc in range(DC):
                nc.tensor.matmul(st_s[:, :nt], ones128, rb[:, dc, :],
                                 start=dc == 0, stop=dc == DC - 1)
            for dc in range(DC):
                nc.tensor.matmul(st_q[:, :nt], ones128b,
                                 sqB[:, dc, c0:c0 + nt],
                                 start=dc == 0, stop=dc == DC - 1)
            mrow = rowp.tile([1, nt], F32, tag="mrow")
            nc.vector.tensor_scalar_mul(mrow, st_s[:, :nt], inv_d)
            msq = rowp.tile([1, nt], F32, tag="msq")
            nc.vector.tensor_mul(msq, mrow, mrow)
            wrow = rowp.tile([1, nt], F32, tag="wrow")
            # wrow = st_q/D - m^2
            nc.vector.scalar_tensor_tensor(out=wrow, in0=st_q[:, :nt],
                                           scalar=inv_d, in1=msq,
                                           op0=OP.mult, op1=OP.subtract)
            srow = rowp.tile([1, nt], F32R, tag="srow")
            nc.scalar.activation(srow, wrow, AF.Sqrt, bias=epsT[0:1, :])
            nc.vector.reciprocal(srow, srow)
            # nms = -m*s
            nms = rowp.tile([1, nt], F32R, tag="nms")
            nc.vector.scalar_tensor_tensor(out=nms, in0=mrow, scalar=-1.0,
                                           in1=srow, op0=OP.mult, op1=OP.mult)
            bc = evp.tile([P, 2, 512], F32, space="PSUM", tag="ev")
            nc.tensor.matmul(bc[:, 0, :nt], ones8[0:1, :], srow,
                             start=True, stop=True)
            nc.tensor.matmul(bc[:, 1, :nt], ones8[0:1, :], nms,
                             start=True, stop=True)
            u = rp_.tile([P, DC, nt], F32, tag="u")
            nc.vector.tensor_mul(u, rb,
                                 bc[:, 0:1, :nt].broadcast_to([P, DC, nt]))
            nc.vector.tensor_add(u, u,
                                 bc[:, 1:2, :nt].broadcast_to([P, DC, nt]))
            # xb16 first (feeds the next matmuls: critical path), f32 x after
            if bf16_only or write_bf16:
                for dc in range(DC):
                    nc.scalar.activation(
                        out=xb16[:, dc, c0:c0 + nt], in_=u[:, dc, :],
                        func=AF.Identity,
                        bias=b_col[:, dc:dc + 1], scale=g_col[:, dc:dc + 1])
            if not bf16_only:
                for dc in range(DC):
                    nc.vector.tensor_scalar(
                        out=xn[:, dc, c0:c0 + nt], in0=u[:, dc, :],
                        scalar1=g_col[:, dc:dc + 1],
                        scalar2=b_col[:, dc:dc + 1],
                        op0=OP.mult, op1=OP.add)

        def mm1_tile(r, hc, g8, w1_sb, c0, nt):
            hp = hpp.tile([P, 512], F32, space="PSUM", tag="hp")
            for kg, sh in enumerate((0, -1, 1)):
                for kd in range(DC):
                    kc = kg * DC + kd
                    mmb(hp[:, :nt], w1_sb[:, kc, :],
                        xb16[:, kd, c0 + sh:c0 + sh + nt],
                        kc == 0, kc == KC - 1)
            nc.scalar.activation(g8[:, hc, c0:c0 + nt], hp[:, :nt], AF.Gelu,
                                 bias=b1_sb[:, r, hc:hc + 1])
            nc.vector.tensor_mul(g8[:, hc, c0:c0 + nt],
                                 g8[:, hc, c0:c0 + nt],
                                 rwB[:, r, c0:c0 + nt])

        pre_g8 = None  # next step's r0 g8 (hc0 pre-filled during LN)
        for s in range(S):
            xc, xn = (xA, xB) if s % 2 == 0 else (xB, xA)
            tiles = cfg.step_tiles(s)
            # evolved accumulates in SBUF across rules; weights stream once
            evs = evsp.tile([P, DC, WBUF], F32, tag="evs")
            for r in range(R):
                if r == 0 and pre_g8 is not None:
                    g8, hc_lo = pre_g8, 1
                else:
                    g8 = g8p.tile([P, HC, WBUF], BF16, tag="g8", name="g8")
                    hc_lo = 0
                for hc in range(hc_lo, HC):
                    w1_sb = wp.tile([P, KC, P], BF16, tag="w1")
                    nc.sync.dma_start(out=w1_sb, in_=w1t[r, hc])
                    for (c0, nt) in tiles:
                        mm1_tile(r, hc, g8, w1_sb, c0, nt)
                w2r = w2p.tile([P, HC, DC, P], BF16, tag="w2")
                nc.scalar.dma_start(out=w2r, in_=w2t[r])
                last = r == R - 1
                for (c0, nt) in tiles:
                    ev = evp.tile([P, DC, 512], F32, space="PSUM", tag="ev")
                    if r == 0:  # seed: sum_r rw[t,r]*b2[r,d]
                        for dc in range(DC):
                            mmb(ev[:, dc, :nt], b2_sb[:, dc, :],
                                rwT[:, c0:c0 + nt], True, False)
                    for hc in range(HC):
                        for dc in range(DC):
                            mmb(ev[:, dc, :nt], w2r[:, hc, dc, :],
                                g8[:, hc, c0:c0 + nt],
                                r > 0 and hc == 0, hc == HC - 1)
                    if r == 0:
                        nc.vector.tensor_copy(evs[:, :, c0:c0 + nt],
                                              ev[:, :, :nt])
                    else:
                        nc.vector.tensor_add(evs[:, :, c0:c0 + nt],
                                             evs[:, :, c0:c0 + nt],
                                             ev[:, :, :nt])
                    if last:
                        # rb = x + evolved and its square, in the mm2 shadow
                        nc.vector.tensor_add(rbF[:, :, c0:c0 + nt],
                                             xc[:, :, c0:c0 + nt],
                                             evs[:, :, c0:c0 + nt])
                        nc.scalar.square(sqB[:, :, c0:c0 + nt],
                                         rbF[:, :, c0:c0 + nt])
            if s < S - 1:
                # LN tiles = next step's mm1 read windows (+/-1 col), so each
                # interleaved next-step mm1 tile depends only on the LN tile
                # emitted just before it
                nxt = cfg.step_tiles(s + 1)
                pre_g8 = g8p.tile([P, HC, WBUF], BF16, tag="g8")
                w1_sb = wp.tile([P, KC, P], BF16, tag="w1")
                nc.sync.dma_start(out=w1_sb, in_=w1t[0, 0])
                for (c0, nt) in nxt:
                    ln_core(rbF, c0 - 1, nt + 2, ng_sb[:, s, :],
                            nbv_sb[:, s, :], xn)
                    mm1_tile(0, 0, pre_g8, w1_sb, c0, nt)
            else:
                # last CA step LN (f32 only) + final LN (bf16 only) per tile;
                # head consumes xb16 afterwards
                fins = cfg.final_tiles()
                done4 = []
                for j, (c0, nt) in enumerate(fins):
                    ln_core(rbF, c0, nt, ng_sb[:, s, :], nbv_sb[:, s, :],
                            xn, write_bf16=False)
                    nc.scalar.square(sqB[:, :, c0:c0 + nt],
                                     xn[:, :, c0:c0 + nt])
                    done4.append((c0, nt))
                    if j >= 1:
                        (fc0, fnt) = done4.pop(0)
                        ln_core(xn, fc0, fnt, lg_sb[:, 0, :], lb_sb[:, 0, :],
                                None, bf16_only=True)
                for (fc0, fnt) in done4:
                    ln_core(xn, fc0, fnt, lg_sb[:, 0, :], lb_sb[:, 0, :],
                            None, bf16_only=True)

    # ---- head ----------------------------------------------------------
    GW = cfg.VGW * cfg.VPG  # 2000 vocab cols per group
    with tc.tile_pool(name="hwp", bufs=2) as hwp, \
         tc.tile_pool(name="obp", bufs=3) as obp, \
         tc.tile_pool(name="outp", bufs=2, space="PSUM") as outp:
        for vg in range(cfg.NVG):
            # vtile stride padded to 512 so each slice is bank-aligned
            hw_sb = hwp.tile([P, DC, cfg.VPG, 512], BF16, tag="hw")
            for dc in range(DC):
                nc.sync.dma_start(out=hw_sb[:, dc, :, :cfg.VGW], in_=bass.AP(
                    headw.tensor, vg * GW + dc * P * cfg.V,
                    [[cfg.V, P], [cfg.VGW, cfg.VPG], [1, cfg.VGW]]))
            for tk in range(cfg.n_tok_chunks):
                c = cfg.own_col0 + tk * P
                po = outp.tile([P, cfg.VPG, 512], F32, space="PSUM", tag="po")
                for dc in range(DC):
                    for vt in range(cfg.VPG):
                        mmb(po[:, vt, :cfg.VGW], xb16[:, dc, c:c + P],
                            hw_sb[:, dc, vt, :cfg.VGW],
                            dc == 0, dc == DC - 1)
                ob = obp.tile([P, cfg.VPG, cfg.VGW], BF16, tag="ob")
                half = cfg.VPG // 2
                nc.vector.tensor_copy(ob[:, :half, :], po[:, :half, :cfg.VGW])
                nc.scalar.copy(ob[:, half:, :], po[:, half:, :cfg.VGW])
                nc.sync.dma_start(
                    out=out[tk * P:(tk + 1) * P, vg * GW:(vg + 1) * GW],
                    in_=ob)


# ---- host-side sharding / unsharding -----------------------------------

def _bf16(a):
    import ml_dtypes
    return np.ascontiguousarray(np.asarray(a, np.float32).astype(
        ml_dtypes.bfloat16))


def shard_inputs(cfg: Cfg, tokens, gate_signal, embed, pos_embed, rule_w1,
                 rule_b1, rule_w2, rule_b2, sel_w, sel_b, norm_g, norm_b,
                 lnf_g, lnf_b, head_w, n_cores=8):
    D, R, T = cfg.D, cfg.R, cfg.T
    w1t = _bf16(np.asarray(rule_w1, np.float32)
                .reshape(R, cfg.KC, P, cfg.HC, P).transpose(0, 3, 2, 1, 4))
    w2t = _bf16(np.asarray(rule_w2, np.float32)
                .reshape(R, cfg.HC, P, cfg.DC, P).transpose(0, 2, 1, 3, 4))
    shared = {
        "embed": np.ascontiguousarray(embed, np.float32),
        "w1t": w1t,
        "b1": np.ascontiguousarray(rule_b1, np.float32),
        "w2t": w2t,
        "b2": _bf16(rule_b2),
        "selw": np.ascontiguousarray(sel_w, np.float32),
        "selb": np.ascontiguousarray(sel_b, np.float32).reshape(1, R),
        "ng": np.ascontiguousarray(norm_g, np.float32),
        "nb": np.ascontiguousarray(norm_b, np.float32),
        "lg": np.ascontiguousarray(lnf_g, np.float32).reshape(1, D),
        "lb": np.ascontiguousarray(lnf_b, np.float32).reshape(1, D),
        "headw": _bf16(head_w),
        "ones1": np.ones((P, 1), np.float32),
        "ones8d": np.ones((8, P), np.float32),
    }
    halves = T // cfg.own
    in_maps = []
    for c in range(n_cores):
        b, h = divmod(c, halves)
        t0 = h * cfg.own
        w = np.arange(t0 - cfg.halo, t0 - cfg.halo + cfg.WBUF) % T
        toks_win = np.asarray(tokens)[b, w].astype(np.int32)
        m = dict(shared)
        m["toksT"] = np.ascontiguousarray(toks_win.reshape(cfg.NT9, P).T)
        m["gate"] = np.ascontiguousarray(
            np.asarray(gate_signal, np.float32)[0, w, :])
        m["pos"] = np.ascontiguousarray(np.asarray(pos_embed, np.float32)[w, :])
        in_maps.append(m)
    return in_maps


def unshard_output(cfg: Cfg, results, n_cores=8):
    halves = cfg.T // cfg.own
    out = np.empty((cfg.B, cfg.T, cfg.V), np.float32)
    for c in range(n_cores):
        b, h = divmod(c, halves)
        out[b, h * cfg.own:(h + 1) * cfg.own, :] = \
            np.asarray(results[c]["out"]).astype(np.float32)
    return out


_NC_CACHE = {}


def kernel(**inputs):
    cfg = Cfg()
    if "full" not in _NC_CACHE:
        _NC_CACHE["full"] = build_nc(cfg)
    nc = _NC_CACHE["full"]
    in_maps = shard_inputs(cfg, **{k: np.asarray(v) for k, v in inputs.items()})
    res = run_bass_kernel_spmd(nc, in_maps, core_ids=list(range(8)))
    return unshard_output(cfg, res.results)


# revision 30
# speedup vs baseline: 1.0557x; 1.0091x over previous
"""Trainium2 Bass kernel for nn_BenchCADecoder (cellular-automaton decoder).

Model: x = embed[tokens]+pos; rw = softmax(gate*1e-3 @ sel_w + sel_b) (step
invariant); 5 CA steps of x = LN(x + sum_r rw[t,r] * MLP_r([x, roll(x,1),
roll(x,-1)])); out = LN_f(x) @ head_w.

Sharding: pure data-parallel over (batch, T-half): 8 cores x 1024 tokens,
each with a circular halo of 5 tokens per side so the 5 neighbor-coupled
steps need zero inter-core communication (window shrinks by 1/side/step).

On-chip layout: x kept transposed [D, tok] so roll() is a free-dim slice
shift and both MLP matmuls run with the contraction on partitions. All MLP
and head matmuls in bf16 (weights pre-cast on host, x cast on-chip once per
step): bf16 enables fast-weight-load so the per-matmul LDWEIGHTS hides
under the previous matmul's stream, and it halves weight DMA. The residual
x stream and all LayerNorm math stay f32. The rule-weighted sum is folded
into PSUM accumulation of the second matmul by pre-scaling gelu outputs
with broadcast rule weights. LayerNorm stats (partition-dim reductions) via
ones-vector matmuls in f32r. Head output is written bf16 and widened to
f32 on host (halves the 131MB/core logit write).
"""

import sys
from contextlib import ExitStack

import numpy as np

sys.path.insert(0, "/opt/trn_rl_repo")

import concourse.bacc as bacc
import concourse.bass as bass
import concourse.mybir as mybir
import concourse.tile as tile
from concourse.bass import IndirectOffsetOnAxis
from concourse.bass_utils import run_bass_kernel_spmd
from concourse.masks import make_identity

F32 = mybir.dt.float32
F32R = mybir.dt.float32r
BF16 = mybir.dt.bfloat16
I32 = mybir.dt.int32
AF = mybir.ActivationFunctionType
OP = mybir.AluOpType

P = 128


class Cfg:
    def __init__(self, D=512, R=8, V=32000, T=2048, B=4, steps=5, own=1024,
                 halo=5, eps=1e-5, gate_scale=1e-3, newton=False):
        self.D, self.R, self.V, self.T, self.B = D, R, V, T, B
        self.steps, self.own, self.halo = steps, own, halo
        self.eps, self.gate_scale = eps, gate_scale
        self.newton = newton
        self.DC = D // P                 # d chunks
        self.HID = 2 * D
        self.HC = self.HID // P          # hidden chunks
        self.KC = 3 * self.DC            # contraction chunks for mm1
        self.WIN = own + 2 * halo        # gathered token window (1034)
        self.NT9 = (self.WIN + P - 1) // P   # gather tiles (9)
        self.WBUF = self.NT9 * P         # x buffer cols (1152)
        self.own_col0 = halo             # first owned col in x buffer
        self.n_tok_chunks = own // P     # head token chunks (8 x 128)
        # head vocab tiling: 16 groups x 4 tiles x 500 cols = 32000
        self.VGW = 500
        self.VPG = 4
        self.NVG = V // (self.VGW * self.VPG)

    def step_tiles(self, s):
        """Output-window tiles for CA step s: [(col0, width)] x3."""
        W = self.own + 2 * (self.steps - 1 - s)
        lo = s + 1
        return self._split3(lo, W)

    def final_tiles(self):
        return self._split3(self.own_col0, self.own)

    @staticmethod
    def _split3(lo, W):
        # three even-width tiles (f32r matmuls require even free sizes)
        w = ((W + 5) // 6) * 2
        return [(lo, w), (lo + w, w), (lo + 2 * w, W - 2 * w)]


def _r(ap):
    """View an f32 AP as float32r for PE consumption."""
    return ap.bitcast(F32R)


def build_nc(cfg: Cfg, num_devices=8):
    """Build the single-core (SPMD) Bass module."""
    nc = bacc.Bacc("TRN2", target_bir_lowering=False, debug=False,
                   num_devices=num_devices)
    D, R, V, HC, KC = cfg.D, cfg.R, cfg.V, cfg.HC, cfg.KC
    S = cfg.steps

    # ---- DRAM I/O ------------------------------------------------------
    # x0 = embed[tokens] + pos, pre-transposed on host ([d, tok] layout)
    xin = nc.dram_tensor("xin", [P, cfg.DC, cfg.WBUF], F32R,
                         kind="ExternalInput").ap()
    xinb = nc.dram_tensor("xinb", [P, cfg.DC, cfg.WBUF], BF16,
                          kind="ExternalInput").ap()
    # rw = softmax(gate*scale @ sel_w + sel_b), computed on host
    rwin = nc.dram_tensor("rwin", [R, cfg.WBUF], BF16,
                          kind="ExternalInput").ap()
    w1t = nc.dram_tensor("w1t", [R, HC, P, KC, P], BF16, kind="ExternalInput").ap()
    b1 = nc.dram_tensor("b1", [R, 2 * D], F32, kind="ExternalInput").ap()
    w2t = nc.dram_tensor("w2t", [R, P, HC, cfg.DC, P], BF16,
                         kind="ExternalInput").ap()
    b2 = nc.dram_tensor("b2", [R, D], BF16, kind="ExternalInput").ap()
    ng = nc.dram_tensor("ng", [S, D], F32, kind="ExternalInput").ap()
    nb_ = nc.dram_tensor("nb", [S, D], F32, kind="ExternalInput").ap()
    lg = nc.dram_tensor("lg", [1, D], F32, kind="ExternalInput").ap()
    lb = nc.dram_tensor("lb", [1, D], F32, kind="ExternalInput").ap()
    headw = nc.dram_tensor("headw", [D, V], BF16, kind="ExternalInput").ap()
    ones1_d = nc.dram_tensor("ones1", [P, 1], F32R, kind="ExternalInput").ap()
    ones8_d = nc.dram_tensor("ones8d", [8, P], F32R, kind="ExternalInput").ap()
    out = nc.dram_tensor("out", [cfg.own, V], BF16, kind="ExternalOutput").ap()

    with ExitStack() as ctx:
        ctx.enter_context(nc.allow_low_precision(reason="bf16 mms by design"))
        tc = ctx.enter_context(tile.TileContext(nc))
        _emit(ctx, tc, cfg, xin, xinb, rwin, w1t, b1, w2t, b2,
              ng, nb_, lg, lb, headw, out, ones1_d, ones8_d)
    nc.compile()
    return nc


def _emit(ctx, tc, cfg, xin, xinb, rwin, w1t, b1, w2t, b2,
          ng, nb_, lg, lb, headw, out, ones1_d, ones8_d):
    nc = tc.nc
    D, R, DC, HC, KC = cfg.D, cfg.R, cfg.DC, cfg.HC, cfg.KC
    S, WBUF, NT9 = cfg.steps, cfg.WBUF, cfg.NT9

    def mmb(o, lh, rh, start, stop):
        nc.tensor.matmul(o, lh, rh, start=start, stop=stop)

    # ---- persistent SBUF ----------------------------------------------
    persist = ctx.enter_context(tc.tile_pool(name="persist", bufs=1))
    xA = persist.tile([P, DC, WBUF], F32R, name="xA")
    xB = persist.tile([P, DC, WBUF], F32R, name="xB")
    xb16 = persist.tile([P, DC, WBUF], BF16, name="xb16")
    rbF = persist.tile([P, DC, WBUF], F32R, name="rbF")   # x + evolved
    sqB = persist.tile([P, DC, WBUF], BF16, name="sqB")   # (x + evolved)^2
    ones128b = persist.tile([P, 1], BF16, name="ones128b")
    epsT = persist.tile([P, 1], F32, name="epsT")
    rwB = persist.tile([P, R, WBUF], BF16, name="rwB")   # bcast rule weights
    rwT = persist.tile([R, WBUF], BF16, name="rwT")      # rw [r, tok]
    ones128 = persist.tile([P, 1], F32R, name="ones128")
    ones8 = persist.tile([8, P], F32R, name="ones8")
    b1_sb = persist.tile([P, R, HC], F32, name="b1_sb")
    b2_sb = persist.tile([R, DC, P], BF16, name="b2_sb")
    ng_sb = persist.tile([P, S, DC], F32, name="ng_sb")
    nbv_sb = persist.tile([P, S, DC], F32, name="nbv_sb")
    lg_sb = persist.tile([P, 1, DC], F32, name="lg_sb")
    lb_sb = persist.tile([P, 1, DC], F32, name="lb_sb")

    nc.sync.dma_start(out=ones128, in_=ones1_d)
    nc.sync.dma_start(out=ones8[0:8, :], in_=ones8_d)
    nc.vector.memset(ones128b, 1.0)
    nc.vector.memset(epsT, cfg.eps)

    nc.sync.dma_start(out=b1_sb, in_=bass.AP(
        b1.tensor, 0, [[1, P], [2 * D, R], [P, HC]]))
    nc.sync.dma_start(out=b2_sb, in_=bass.AP(
        b2.tensor, 0, [[D, R], [P, DC], [1, P]]))
    nc.sync.dma_start(out=ng_sb, in_=bass.AP(
        ng.tensor, 0, [[1, P], [D, S], [P, DC]]))
    nc.sync.dma_start(out=nbv_sb, in_=bass.AP(
        nb_.tensor, 0, [[1, P], [D, S], [P, DC]]))
    nc.sync.dma_start(out=lg_sb, in_=bass.AP(
        lg.tensor, 0, [[1, P], [D, 1], [P, DC]]))
    nc.sync.dma_start(out=lb_sb, in_=bass.AP(
        lb.tensor, 0, [[1, P], [D, 1], [P, DC]]))

    # ---- setup: x0 / rw precomputed on host, straight DMAs -------------
    # xb16 first: step-0 mm1 depends only on it (+ w1), starts immediately
    nc.sync.dma_start(out=xb16, in_=xinb)
    nc.sync.dma_start(out=rwT, in_=rwin)
    # broadcast rw rows across partitions: rwB[p, r, c] = rw[tok c, r]
    for r in range(R):
        nc.sync.dma_start(
            out=rwB[:, r, :],
            in_=bass.AP(rwin.tensor, r * WBUF, [[0, P], [1, WBUF]]))
    nc.scalar.dma_start(out=xA, in_=xin)

    # ---- CA steps ------------------------------------------------------
    with tc.tile_pool(name="w1p", bufs=3) as wp, \
         tc.tile_pool(name="w2p", bufs=2) as w2p, \
         tc.tile_pool(name="g8p", bufs=2) as g8p, \
         tc.tile_pool(name="evsp", bufs=1) as evsp, \
         tc.tile_pool(name="rbp", bufs=1) as rp_, \
         tc.tile_pool(name="rowp", bufs=1) as rowp, \
         tc.tile_pool(name="evp", bufs=1, space="PSUM") as evp, \
         tc.tile_pool(name="hpp", bufs=2, space="PSUM") as hpp, \
         tc.tile_pool(name="stp", bufs=1, space="PSUM") as stp:

        def ln_core(rb_src, c0, nt, g_col, b_col, xn, bf16_only=False,
                    write_bf16=True):
            """LN over cols [c0, c0+nt) from rb_src (+ sqB) -> xn / xb16.

            rb_src and sqB[:, :, c0:c0+nt] must already hold x+ev and its
            square (computed in the mm2 shadow).
            """
            inv_d = 1.0 / D
            rb = rb_src[:, :, c0:c0 + nt]
            st_s = stp.tile([1, 512], F32, space="PSUM", tag="sts")
            st_q = stp.tile([1, 512], F32, space="PSUM", tag="stq")
            for dc in range(DC):
                nc.tensor.matmul(st_s[:, :nt], ones128, rb[:, dc, :],
                                 start=dc == 0, stop=dc == DC - 1)
            for dc in range(DC):
                nc.tensor.matmul(st_q[:, :nt], ones128b,
                                 sqB[:, dc, c0:c0 + nt],
                                 start=dc == 0, stop=dc == DC - 1)
            mrow = rowp.tile([1, nt], F32, tag="mrow")
            nc.vector.tensor_scalar_mul(mrow, st_s[:, :nt], inv_d)
            msq = rowp.tile([1, nt], F32, tag="msq")
            nc.vector.tensor_mul(msq, mrow, mrow)
            wrow = rowp.tile([1, nt], F32, tag="wrow")
            # wrow = st_q/D - m^2
            nc.vector.scalar_tensor_tensor(out=wrow, in0=st_q[:, :nt],
                                           scalar=inv_d, in1=msq,
                                           op0=OP.mult, op1=OP.subtract)
            srow = rowp.tile([1, nt], F32R, tag="srow")
            nc.scalar.activation(srow, wrow, AF.Sqrt, bias=epsT[0:1, :])
            nc.vector.reciprocal(srow, srow)
            # nms = -m*s
            nms = rowp.tile([1, nt], F32R, tag="nms")
            nc.vector.scalar_tensor_tensor(out=nms, in0=mrow, scalar=-1.0,
                                           in1=srow, op0=OP.mult, op1=OP.mult)
            bc = evp.tile([P, 2, 512], F32, space="PSUM", tag="ev")
            nc.tensor.matmul(bc[:, 0, :nt], ones8[0:1, :], srow,
                             start=True, stop=True)
            nc.tensor.matmul(bc[:, 1, :nt], ones8[0:1, :], nms,
                             start=True, stop=True)
            u = rp_.tile([P, DC, nt], F32, tag="u")
            nc.vector.tensor_mul(u, rb,
                                 bc[:, 0:1, :nt].broadcast_to([P, DC, nt]))
            nc.vector.tensor_add(u, u,
                                 bc[:, 1:2, :nt].broadcast_to([P, DC, nt]))
            # xb16 first (feeds the next matmuls: critical path), f32 x after
            if bf16_only or write_bf16:
                for dc in range(DC):
                    nc.scalar.activation(
                        out=xb16[:, dc, c0:c0 + nt], in_=u[:, dc, :],
                        func=AF.Identity,
                        bias=b_col[:, dc:dc + 1], scale=g_col[:, dc:dc + 1])
            if not bf16_only:
                for dc in range(DC):
                    nc.vector.tensor_scalar(
                        out=xn[:, dc, c0:c0 + nt], in0=u[:, dc, :],
                        scalar1=g_col[:, dc:dc + 1],
                        scalar2=b_col[:, dc:dc + 1],
                        op0=OP.mult, op1=OP.add)

        def mm1_tile(r, hc, g8, w1_sb, c0, nt):
            hp = hpp.tile([P, 512], F32, space="PSUM", tag="hp")
            for kg, sh in enumerate((0, -1, 1)):
                for kd in range(DC):
                    kc = kg * DC + kd
                    mmb(hp[:, :nt], w1_sb[:, kc, :],
                        xb16[:, kd, c0 + sh:c0 + sh + nt],
                        kc == 0, kc == KC - 1)
            nc.scalar.activation(g8[:, hc, c0:c0 + nt], hp[:, :nt], AF.Gelu,
                                 bias=b1_sb[:, r, hc:hc + 1])
            nc.vector.tensor_mul(g8[:, hc, c0:c0 + nt],
                                 g8[:, hc, c0:c0 + nt],
                                 rwB[:, r, c0:c0 + nt])

        pre_g8 = None  # next step's r0 g8 (hc0 pre-filled during LN)
        for s in range(S):
            xc, xn = (xA, xB) if s % 2 == 0 else (xB, xA)
            tiles = cfg.step_tiles(s)
            # evolved accumulates in SBUF across rules; weights stream once
            evs = evsp.tile([P, DC, WBUF], F32, tag="evs")
            for r in range(R):
                if r == 0 and pre_g8 is not None:
                    g8, hc_lo = pre_g8, 1
                else:
                    g8 = g8p.tile([P, HC, WBUF], BF16, tag="g8", name="g8")
                    hc_lo = 0
                for hc in range(hc_lo, HC):
                    w1_sb = wp.tile([P, KC, P], BF16, tag="w1")
                    nc.sync.dma_start(out=w1_sb, in_=w1t[r, hc])
                    for (c0, nt) in tiles:
                        mm1_tile(r, hc, g8, w1_sb, c0, nt)
                w2r = w2p.tile([P, HC, DC, P], BF16, tag="w2")
                nc.scalar.dma_start(out=w2r, in_=w2t[r])
                last = r == R - 1
                for (c0, nt) in tiles:
                    ev = evp.tile([P, DC, 512], F32, space="PSUM", tag="ev")
                    if r == 0:  # seed: sum_r rw[t,r]*b2[r,d]
                        for dc in range(DC):
                            mmb(ev[:, dc, :nt], b2_sb[:, dc, :],
                                rwT[:, c0:c0 + nt], True, False)
                    for hc in range(HC):
                        for dc in range(DC):
                            mmb(ev[:, dc, :nt], w2r[:, hc, dc, :],
                                g8[:, hc, c0:c0 + nt],
                                r > 0 and hc == 0, hc == HC - 1)
                    if r == 0:
                        nc.vector.tensor_copy(evs[:, :, c0:c0 + nt],
                                              ev[:, :, :nt])
                    else:
                        nc.vector.tensor_add(evs[:, :, c0:c0 + nt],
                                             evs[:, :, c0:c0 + nt],
                                             ev[:, :, :nt])
                    if last:
                        # rb = x + evolved and its square, in the mm2 shadow
                        nc.vector.tensor_add(rbF[:, :, c0:c0 + nt],
                                             xc[:, :, c0:c0 + nt],
                                             evs[:, :, c0:c0 + nt])
                        nc.scalar.square(sqB[:, :, c0:c0 + nt],
                                         rbF[:, :, c0:c0 + nt])
            if s < S - 1:
                # LN tiles = next step's mm1 read windows (+/-1 col), so each
                # interleaved next-step mm1 tile depends only on the LN tile
                # emitted just before it
                nxt = cfg.step_tiles(s + 1)
                pre_g8 = g8p.tile([P, HC, WBUF], BF16, tag="g8")
                w1_sb = wp.tile([P, KC, P], BF16, tag="w1")
                nc.sync.dma_start(out=w1_sb, in_=w1t[0, 0])
                for (c0, nt) in nxt:
                    ln_core(rbF, c0 - 1, nt + 2, ng_sb[:, s, :],
                            nbv_sb[:, s, :], xn)
                    mm1_tile(0, 0, pre_g8, w1_sb, c0, nt)
            else:
                # last CA step LN (f32 only) + final LN (bf16 only) per tile;
                # head consumes xb16 afterwards
                fins = cfg.final_tiles()
                done4 = []
                for j, (c0, nt) in enumerate(fins):
                    ln_core(rbF, c0, nt, ng_sb[:, s, :], nbv_sb[:, s, :],
                            xn, write_bf16=False)
                    nc.scalar.square(sqB[:, :, c0:c0 + nt],
                                     xn[:, :, c0:c0 + nt])
                    done4.append((c0, nt))
                    if j >= 1:
                        (fc0, fnt) = done4.pop(0)
                        ln_core(xn, fc0, fnt, lg_sb[:, 0, :], lb_sb[:, 0, :],
                                None, bf16_only=True)
                for (fc0, fnt) in done4:
                    ln_core(xn, fc0, fnt, lg_sb[:, 0, :], lb_sb[:, 0, :],
                            None, bf16_only=True)

    # ---- head ----------------------------------------------------------
    GW = cfg.VGW * cfg.VPG  # 2000 vocab cols per group
    with tc.tile_pool(name="hwp", bufs=2) as hwp, \
         tc.tile_pool(name="obp", bufs=3) as obp, \
         tc.tile_pool(name="outp", bufs=2, space="PSUM") as outp:
        for vg in range(cfg.NVG):
            # vtile stride padded to 512 so each slice is bank-aligned
            hw_sb = hwp.tile([P, DC, cfg.VPG, 512], BF16, tag="hw")
            for dc in range(DC):
                nc.sync.dma_start(out=hw_sb[:, dc, :, :cfg.VGW], in_=bass.AP(
                    headw.tensor, vg * GW + dc * P * cfg.V,
                    [[cfg.V, P], [cfg.VGW, cfg.VPG], [1, cfg.VGW]]))
            for tk in range(cfg.n_tok_chunks):
                c = cfg.own_col0 + tk * P
                po = outp.tile([P, cfg.VPG, 512], F32, space="PSUM", tag="po")
                for dc in range(DC):
                    for vt in range(cfg.VPG):
                        mmb(po[:, vt, :cfg.VGW], xb16[:, dc, c:c + P],
                            hw_sb[:, dc, vt, :cfg.VGW],
                            dc == 0, dc == DC - 1)
                ob = obp.tile([P, cfg.VPG, cfg.VGW], BF16, tag="ob")
                half = cfg.VPG // 2
                nc.vector.tensor_copy(ob[:, :half, :], po[:, :half, :cfg.VGW])
                nc.scalar.copy(ob[:, half:, :], po[:, half:, :cfg.VGW])
                nc.sync.dma_start(
                    out=out[tk * P:(tk + 1) * P, vg * GW:(vg + 1) * GW],
                    in_=ob)


# ---- host-side sharding / unsharding -----------------------------------

def _bf16(a):
    import ml_dtypes
    return np.ascontiguousarray(np.asarray(a, np.float32).astype(
        ml_dtypes.bfloat16))


def shard_inputs(cfg: Cfg, tokens, gate_signal, embed, pos_embed, rule_w1,
                 rule_b1, rule_w2, rule_b2, sel_w, sel_b, norm_g, norm_b,
                 lnf_g, lnf_b, head_w, n_cores=8):
    D, R, T = cfg.D, cfg.R, cfg.T
    w1t = _bf16(np.asarray(rule_w1, np.float32)
                .reshape(R, cfg.KC, P, cfg.HC, P).transpose(0, 3, 2, 1, 4))
    w2t = _bf16(np.asarray(rule_w2, np.float32)
                .reshape(R, cfg.HC, P, cfg.DC, P).transpose(0, 2, 1, 3, 4))
    shared = {
        "w1t": w1t,
        "b1": np.ascontiguousarray(rule_b1, np.float32),
        "w2t": w2t,
        "b2": _bf16(rule_b2),
        "ng": np.ascontiguousarray(norm_g, np.float32),
        "nb": np.ascontiguousarray(norm_b, np.float32),
        "lg": np.ascontiguousarray(lnf_g, np.float32).reshape(1, D),
        "lb": np.ascontiguousarray(lnf_b, np.float32).reshape(1, D),
        "headw": _bf16(head_w),
        "ones1": np.ones((P, 1), np.float32),
        "ones8d": np.ones((8, P), np.float32),
    }
    # rule selector: softmax(gate*scale @ sel_w + sel_b) -> [T, R]
    logits = (np.asarray(gate_signal, np.float32)[0] * cfg.gate_scale
              ) @ np.asarray(sel_w, np.float32) + np.asarray(sel_b, np.float32)
    logits -= logits.max(axis=-1, keepdims=True)
    e = np.exp(logits)
    rw = e / e.sum(axis=-1, keepdims=True)          # [T, R]
    embed = np.asarray(embed, np.float32)
    pos = np.asarray(pos_embed, np.float32)
    halves = T // cfg.own
    in_maps = []
    for c in range(n_cores):
        b, h = divmod(c, halves)
        t0 = h * cfg.own
        w = np.arange(t0 - cfg.halo, t0 - cfg.halo + cfg.WBUF) % T
        x0 = embed[np.asarray(tokens)[b, w]] + pos[w]      # [WBUF, D]
        x0t = np.ascontiguousarray(
            x0.T.reshape(cfg.DC, P, cfg.WBUF).transpose(1, 0, 2))
        m = dict(shared)
        m["xin"] = x0t
        m["xinb"] = _bf16(x0t)
        m["rwin"] = _bf16(rw[w, :].T)                      # [R, WBUF]
        in_maps.append(m)
    return in_maps


def unshard_output(cfg: Cfg, results, n_cores=8):
    halves = cfg.T // cfg.own
    out = np.empty((cfg.B, cfg.T, cfg.V), np.float32)
    for c in range(n_cores):
        b, h = divmod(c, halves)
        out[b, h * cfg.own:(h + 1) * cfg.own, :] = \
            np.asarray(results[c]["out"]).astype(np.float32)
    return out


_NC_CACHE = {}


def kernel(**inputs):
    cfg = Cfg()
    if "full" not in _NC_CACHE:
        _NC_CACHE["full"] = build_nc(cfg)
    nc = _NC_CACHE["full"]
    in_maps = shard_inputs(cfg, **{k: np.asarray(v) for k, v in inputs.items()})
    res = run_bass_kernel_spmd(nc, in_maps, core_ids=list(range(8)))
    return unshard_output(cfg, res.results)


# revision 31
# speedup vs baseline: 1.0581x; 1.0023x over previous
"""Trainium2 Bass kernel for nn_BenchCADecoder (cellular-automaton decoder).

Model: x = embed[tokens]+pos; rw = softmax(gate*1e-3 @ sel_w + sel_b) (step
invariant); 5 CA steps of x = LN(x + sum_r rw[t,r] * MLP_r([x, roll(x,1),
roll(x,-1)])); out = LN_f(x) @ head_w.

Sharding: pure data-parallel over (batch, T-half): 8 cores x 1024 tokens,
each with a circular halo of 5 tokens per side so the 5 neighbor-coupled
steps need zero inter-core communication (window shrinks by 1/side/step).

On-chip layout: x kept transposed [D, tok] so roll() is a free-dim slice
shift and both MLP matmuls run with the contraction on partitions. All MLP
and head matmuls in bf16 (weights pre-cast on host, x cast on-chip once per
step): bf16 enables fast-weight-load so the per-matmul LDWEIGHTS hides
under the previous matmul's stream, and it halves weight DMA. The residual
x stream and all LayerNorm math stay f32. The rule-weighted sum is folded
into PSUM accumulation of the second matmul by pre-scaling gelu outputs
with broadcast rule weights. LayerNorm stats (partition-dim reductions) via
ones-vector matmuls in f32r. Head output is written bf16 and widened to
f32 on host (halves the 131MB/core logit write).
"""

import sys
from contextlib import ExitStack

import numpy as np

sys.path.insert(0, "/opt/trn_rl_repo")

import concourse.bacc as bacc
import concourse.bass as bass
import concourse.mybir as mybir
import concourse.tile as tile
from concourse.bass import IndirectOffsetOnAxis
from concourse.bass_utils import run_bass_kernel_spmd
from concourse.masks import make_identity

F32 = mybir.dt.float32
F32R = mybir.dt.float32r
BF16 = mybir.dt.bfloat16
I32 = mybir.dt.int32
AF = mybir.ActivationFunctionType
OP = mybir.AluOpType

P = 128


class Cfg:
    def __init__(self, D=512, R=8, V=32000, T=2048, B=4, steps=5, own=1024,
                 halo=5, eps=1e-5, gate_scale=1e-3, newton=False):
        self.D, self.R, self.V, self.T, self.B = D, R, V, T, B
        self.steps, self.own, self.halo = steps, own, halo
        self.eps, self.gate_scale = eps, gate_scale
        self.newton = newton
        self.DC = D // P                 # d chunks
        self.HID = 2 * D
        self.HC = self.HID // P          # hidden chunks
        self.KC = 3 * self.DC            # contraction chunks for mm1
        self.WIN = own + 2 * halo        # gathered token window (1034)
        self.NT9 = (self.WIN + P - 1) // P   # gather tiles (9)
        self.WBUF = self.NT9 * P         # x buffer cols (1152)
        self.own_col0 = halo             # first owned col in x buffer
        self.n_tok_chunks = own // P     # head token chunks (8 x 128)
        # head vocab tiling: 16 groups x 4 tiles x 500 cols = 32000
        self.VGW = 500
        self.VPG = 4
        self.NVG = V // (self.VGW * self.VPG)

    def step_tiles(self, s):
        """Output-window tiles for CA step s: [(col0, width)] x3."""
        W = self.own + 2 * (self.steps - 1 - s)
        lo = s + 1
        return self._split3(lo, W)

    def final_tiles(self):
        return self._split3(self.own_col0, self.own)

    @staticmethod
    def _split3(lo, W):
        # three even-width tiles (f32r matmuls require even free sizes)
        w = ((W + 5) // 6) * 2
        return [(lo, w), (lo + w, w), (lo + 2 * w, W - 2 * w)]


def _r(ap):
    """View an f32 AP as float32r for PE consumption."""
    return ap.bitcast(F32R)


def build_nc(cfg: Cfg, num_devices=8):
    """Build the single-core (SPMD) Bass module."""
    nc = bacc.Bacc("TRN2", target_bir_lowering=False, debug=False,
                   num_devices=num_devices)
    D, R, V, HC, KC = cfg.D, cfg.R, cfg.V, cfg.HC, cfg.KC
    S = cfg.steps

    # ---- DRAM I/O ------------------------------------------------------
    # x0 = embed[tokens] + pos, pre-transposed on host ([d, tok] layout)
    xin = nc.dram_tensor("xin", [P, cfg.DC, cfg.WBUF], F32R,
                         kind="ExternalInput").ap()
    xinb = nc.dram_tensor("xinb", [P, cfg.DC, cfg.WBUF], BF16,
                          kind="ExternalInput").ap()
    # rw = softmax(gate*scale @ sel_w + sel_b), computed on host
    rwin = nc.dram_tensor("rwin", [R, cfg.WBUF], BF16,
                          kind="ExternalInput").ap()
    w1t = nc.dram_tensor("w1t", [R, HC, P, KC, P], BF16, kind="ExternalInput").ap()
    b1 = nc.dram_tensor("b1", [R, 2 * D], F32, kind="ExternalInput").ap()
    w2t = nc.dram_tensor("w2t", [R, P, HC, cfg.DC, P], BF16,
                         kind="ExternalInput").ap()
    b2 = nc.dram_tensor("b2", [R, D], BF16, kind="ExternalInput").ap()
    ng = nc.dram_tensor("ng", [S, D], F32, kind="ExternalInput").ap()
    nb_ = nc.dram_tensor("nb", [S, D], F32, kind="ExternalInput").ap()
    lg = nc.dram_tensor("lg", [1, D], F32, kind="ExternalInput").ap()
    lb = nc.dram_tensor("lb", [1, D], F32, kind="ExternalInput").ap()
    headw = nc.dram_tensor("headw", [D, V], BF16, kind="ExternalInput").ap()
    ones1_d = nc.dram_tensor("ones1", [P, 1], F32R, kind="ExternalInput").ap()
    ones8_d = nc.dram_tensor("ones8d", [8, P], F32R, kind="ExternalInput").ap()
    out = nc.dram_tensor("out", [cfg.own, V], BF16, kind="ExternalOutput").ap()

    with ExitStack() as ctx:
        ctx.enter_context(nc.allow_low_precision(reason="bf16 mms by design"))
        tc = ctx.enter_context(tile.TileContext(nc))
        _emit(ctx, tc, cfg, xin, xinb, rwin, w1t, b1, w2t, b2,
              ng, nb_, lg, lb, headw, out, ones1_d, ones8_d)
    nc.compile()
    return nc


def _emit(ctx, tc, cfg, xin, xinb, rwin, w1t, b1, w2t, b2,
          ng, nb_, lg, lb, headw, out, ones1_d, ones8_d):
    nc = tc.nc
    D, R, DC, HC, KC = cfg.D, cfg.R, cfg.DC, cfg.HC, cfg.KC
    S, WBUF, NT9 = cfg.steps, cfg.WBUF, cfg.NT9

    def mmb(o, lh, rh, start, stop):
        nc.tensor.matmul(o, lh, rh, start=start, stop=stop)

    # ---- persistent SBUF ----------------------------------------------
    persist = ctx.enter_context(tc.tile_pool(name="persist", bufs=1))
    xA = persist.tile([P, DC, WBUF], F32R, name="xA")
    xB = persist.tile([P, DC, WBUF], F32R, name="xB")
    xb16 = persist.tile([P, DC, WBUF], BF16, name="xb16")
    rbF = persist.tile([P, DC, WBUF], F32R, name="rbF")   # x + evolved
    sqB = persist.tile([P, DC, WBUF], BF16, name="sqB")   # (x + evolved)^2
    ones128b = persist.tile([P, 1], BF16, name="ones128b")
    epsT = persist.tile([P, 1], F32, name="epsT")
    rwB = persist.tile([P, R, WBUF], BF16, name="rwB")   # bcast rule weights
    rwT = persist.tile([R, WBUF], BF16, name="rwT")      # rw [r, tok]
    ones128 = persist.tile([P, 1], F32R, name="ones128")
    ones8 = persist.tile([8, P], F32R, name="ones8")
    b1_sb = persist.tile([P, R, HC], F32, name="b1_sb")
    b2_sb = persist.tile([R, DC, P], BF16, name="b2_sb")
    ng_sb = persist.tile([P, S, DC], F32, name="ng_sb")
    nbv_sb = persist.tile([P, S, DC], F32, name="nbv_sb")
    lg_sb = persist.tile([P, 1, DC], F32, name="lg_sb")
    lb_sb = persist.tile([P, 1, DC], F32, name="lb_sb")

    # ---- setup: x0 / rw precomputed on host, straight DMAs -------------
    # xb16 alone on the sync queue ahead of the w1 stream: step-0 mm1
    # depends only on those two and starts within ~10us
    nc.sync.dma_start(out=xb16, in_=xinb)

    nc.scalar.dma_start(out=ones128, in_=ones1_d)
    nc.scalar.dma_start(out=ones8[0:8, :], in_=ones8_d)
    nc.vector.memset(ones128b, 1.0)
    nc.vector.memset(epsT, cfg.eps)

    nc.scalar.dma_start(out=b1_sb, in_=bass.AP(
        b1.tensor, 0, [[1, P], [2 * D, R], [P, HC]]))
    nc.scalar.dma_start(out=b2_sb, in_=bass.AP(
        b2.tensor, 0, [[D, R], [P, DC], [1, P]]))
    nc.scalar.dma_start(out=ng_sb, in_=bass.AP(
        ng.tensor, 0, [[1, P], [D, S], [P, DC]]))
    nc.scalar.dma_start(out=nbv_sb, in_=bass.AP(
        nb_.tensor, 0, [[1, P], [D, S], [P, DC]]))
    nc.scalar.dma_start(out=lg_sb, in_=bass.AP(
        lg.tensor, 0, [[1, P], [D, 1], [P, DC]]))
    nc.scalar.dma_start(out=lb_sb, in_=bass.AP(
        lb.tensor, 0, [[1, P], [D, 1], [P, DC]]))
    nc.scalar.dma_start(out=rwT, in_=rwin)
    # broadcast rw rows across partitions: rwB[p, r, c] = rw[tok c, r]
    for r in range(R):
        nc.scalar.dma_start(
            out=rwB[:, r, :],
            in_=bass.AP(rwin.tensor, r * WBUF, [[0, P], [1, WBUF]]))
    nc.scalar.dma_start(out=xA, in_=xin)

    # ---- CA steps ------------------------------------------------------
    with tc.tile_pool(name="w1p", bufs=3) as wp, \
         tc.tile_pool(name="w2p", bufs=2) as w2p, \
         tc.tile_pool(name="g8p", bufs=2) as g8p, \
         tc.tile_pool(name="evsp", bufs=1) as evsp, \
         tc.tile_pool(name="rbp", bufs=1) as rp_, \
         tc.tile_pool(name="rowp", bufs=1) as rowp, \
         tc.tile_pool(name="evp", bufs=1, space="PSUM") as evp, \
         tc.tile_pool(name="hpp", bufs=2, space="PSUM") as hpp, \
         tc.tile_pool(name="stp", bufs=1, space="PSUM") as stp:

        def ln_core(rb_src, c0, nt, g_col, b_col, xn, bf16_only=False,
                    write_bf16=True):
            """LN over cols [c0, c0+nt) from rb_src (+ sqB) -> xn / xb16.

            rb_src and sqB[:, :, c0:c0+nt] must already hold x+ev and its
            square (computed in the mm2 shadow).
            """
            inv_d = 1.0 / D
            rb = rb_src[:, :, c0:c0 + nt]
            st_s = stp.tile([1, 512], F32, space="PSUM", tag="sts")
            st_q = stp.tile([1, 512], F32, space="PSUM", tag="stq")
            for dc in range(DC):
                nc.tensor.matmul(st_s[:, :nt], ones128, rb[:, dc, :],
                                 start=dc == 0, stop=dc == DC - 1)
            for dc in range(DC):
                nc.tensor.matmul(st_q[:, :nt], ones128b,
                                 sqB[:, dc, c0:c0 + nt],
                                 start=dc == 0, stop=dc == DC - 1)
            mrow = rowp.tile([1, nt], F32, tag="mrow")
            nc.vector.tensor_scalar_mul(mrow, st_s[:, :nt], inv_d)
            msq = rowp.tile([1, nt], F32, tag="msq")
            nc.vector.tensor_mul(msq, mrow, mrow)
            wrow = rowp.tile([1, nt], F32, tag="wrow")
            # wrow = st_q/D - m^2
            nc.vector.scalar_tensor_tensor(out=wrow, in0=st_q[:, :nt],
                                           scalar=inv_d, in1=msq,
                                           op0=OP.mult, op1=OP.subtract)
            srow = rowp.tile([1, nt], F32R, tag="srow")
            nc.scalar.activation(srow, wrow, AF.Sqrt, bias=epsT[0:1, :])
            nc.vector.reciprocal(srow, srow)
            # nms = -m*s
            nms = rowp.tile([1, nt], F32R, tag="nms")
            nc.vector.scalar_tensor_tensor(out=nms, in0=mrow, scalar=-1.0,
                                           in1=srow, op0=OP.mult, op1=OP.mult)
            bc = evp.tile([P, 2, 512], F32, space="PSUM", tag="ev")
            nc.tensor.matmul(bc[:, 0, :nt], ones8[0:1, :], srow,
                             start=True, stop=True)
            nc.tensor.matmul(bc[:, 1, :nt], ones8[0:1, :], nms,
                             start=True, stop=True)
            u = rp_.tile([P, DC, nt], F32, tag="u")
            nc.vector.tensor_mul(u, rb,
                                 bc[:, 0:1, :nt].broadcast_to([P, DC, nt]))
            nc.vector.tensor_add(u, u,
                                 bc[:, 1:2, :nt].broadcast_to([P, DC, nt]))
            # xb16 first (feeds the next matmuls: critical path), f32 x after
            if bf16_only or write_bf16:
                for dc in range(DC):
                    nc.scalar.activation(
                        out=xb16[:, dc, c0:c0 + nt], in_=u[:, dc, :],
                        func=AF.Identity,
                        bias=b_col[:, dc:dc + 1], scale=g_col[:, dc:dc + 1])
            if not bf16_only:
                for dc in range(DC):
                    nc.vector.tensor_scalar(
                        out=xn[:, dc, c0:c0 + nt], in0=u[:, dc, :],
                        scalar1=g_col[:, dc:dc + 1],
                        scalar2=b_col[:, dc:dc + 1],
                        op0=OP.mult, op1=OP.add)

        def mm1_tile(r, hc, g8, w1_sb, c0, nt):
            hp = hpp.tile([P, 512], F32, space="PSUM", tag="hp")
            for kg, sh in enumerate((0, -1, 1)):
                for kd in range(DC):
                    kc = kg * DC + kd
                    mmb(hp[:, :nt], w1_sb[:, kc, :],
                        xb16[:, kd, c0 + sh:c0 + sh + nt],
                        kc == 0, kc == KC - 1)
            nc.scalar.activation(g8[:, hc, c0:c0 + nt], hp[:, :nt], AF.Gelu,
                                 bias=b1_sb[:, r, hc:hc + 1])
            nc.vector.tensor_mul(g8[:, hc, c0:c0 + nt],
                                 g8[:, hc, c0:c0 + nt],
                                 rwB[:, r, c0:c0 + nt])

        pre_g8 = None  # next step's r0 g8 (hc0 pre-filled during LN)
        for s in range(S):
            xc, xn = (xA, xB) if s % 2 == 0 else (xB, xA)
            tiles = cfg.step_tiles(s)
            # evolved accumulates in SBUF across rules; weights stream once
            evs = evsp.tile([P, DC, WBUF], F32, tag="evs")
            for r in range(R):
                if r == 0 and pre_g8 is not None:
                    g8, hc_lo = pre_g8, 1
                else:
                    g8 = g8p.tile([P, HC, WBUF], BF16, tag="g8", name="g8")
                    hc_lo = 0
                for hc in range(hc_lo, HC):
                    w1_sb = wp.tile([P, KC, P], BF16, tag="w1")
                    nc.sync.dma_start(out=w1_sb, in_=w1t[r, hc])
                    for (c0, nt) in tiles:
                        mm1_tile(r, hc, g8, w1_sb, c0, nt)
                w2r = w2p.tile([P, HC, DC, P], BF16, tag="w2")
                nc.scalar.dma_start(out=w2r, in_=w2t[r])
                last = r == R - 1
                for (c0, nt) in tiles:
                    ev = evp.tile([P, DC, 512], F32, space="PSUM", tag="ev")
                    if r == 0:  # seed: sum_r rw[t,r]*b2[r,d]
                        for dc in range(DC):
                            mmb(ev[:, dc, :nt], b2_sb[:, dc, :],
                                rwT[:, c0:c0 + nt], True, False)
                    for hc in range(HC):
                        for dc in range(DC):
                            mmb(ev[:, dc, :nt], w2r[:, hc, dc, :],
                                g8[:, hc, c0:c0 + nt],
                                r > 0 and hc == 0, hc == HC - 1)
                    if r == 0:
                        nc.vector.tensor_copy(evs[:, :, c0:c0 + nt],
                                              ev[:, :, :nt])
                    else:
                        nc.vector.tensor_add(evs[:, :, c0:c0 + nt],
                                             evs[:, :, c0:c0 + nt],
                                             ev[:, :, :nt])
                    if last:
                        # rb = x + evolved and its square, in the mm2 shadow
                        nc.vector.tensor_add(rbF[:, :, c0:c0 + nt],
                                             xc[:, :, c0:c0 + nt],
                                             evs[:, :, c0:c0 + nt])
                        nc.scalar.square(sqB[:, :, c0:c0 + nt],
                                         rbF[:, :, c0:c0 + nt])
            if s < S - 1:
                # LN tiles = next step's mm1 read windows (+/-1 col), so each
                # interleaved next-step mm1 tile depends only on the LN tile
                # emitted just before it
                nxt = cfg.step_tiles(s + 1)
                pre_g8 = g8p.tile([P, HC, WBUF], BF16, tag="g8")
                w1_sb = wp.tile([P, KC, P], BF16, tag="w1")
                nc.sync.dma_start(out=w1_sb, in_=w1t[0, 0])
                for (c0, nt) in nxt:
                    ln_core(rbF, c0 - 1, nt + 2, ng_sb[:, s, :],
                            nbv_sb[:, s, :], xn)
                    mm1_tile(0, 0, pre_g8, w1_sb, c0, nt)
            else:
                # last CA step LN (f32 only) + final LN (bf16 only) per tile;
                # head consumes xb16 afterwards
                fins = cfg.final_tiles()
                done4 = []
                for j, (c0, nt) in enumerate(fins):
                    ln_core(rbF, c0, nt, ng_sb[:, s, :], nbv_sb[:, s, :],
                            xn, write_bf16=False)
                    nc.scalar.square(sqB[:, :, c0:c0 + nt],
                                     xn[:, :, c0:c0 + nt])
                    done4.append((c0, nt))
                    if j >= 1:
                        (fc0, fnt) = done4.pop(0)
                        ln_core(xn, fc0, fnt, lg_sb[:, 0, :], lb_sb[:, 0, :],
                                None, bf16_only=True)
                for (fc0, fnt) in done4:
                    ln_core(xn, fc0, fnt, lg_sb[:, 0, :], lb_sb[:, 0, :],
                            None, bf16_only=True)

    # ---- head ----------------------------------------------------------
    GW = cfg.VGW * cfg.VPG  # 2000 vocab cols per group
    with tc.tile_pool(name="hwp", bufs=2) as hwp, \
         tc.tile_pool(name="obp", bufs=3) as obp, \
         tc.tile_pool(name="outp", bufs=2, space="PSUM") as outp:
        for vg in range(cfg.NVG):
            # vtile stride padded to 512 so each slice is bank-aligned
            hw_sb = hwp.tile([P, DC, cfg.VPG, 512], BF16, tag="hw")
            for dc in range(DC):
                nc.sync.dma_start(out=hw_sb[:, dc, :, :cfg.VGW], in_=bass.AP(
                    headw.tensor, vg * GW + dc * P * cfg.V,
                    [[cfg.V, P], [cfg.VGW, cfg.VPG], [1, cfg.VGW]]))
            for tk in range(cfg.n_tok_chunks):
                c = cfg.own_col0 + tk * P
                po = outp.tile([P, cfg.VPG, 512], F32, space="PSUM", tag="po")
                for dc in range(DC):
                    for vt in range(cfg.VPG):
                        mmb(po[:, vt, :cfg.VGW], xb16[:, dc, c:c + P],
                            hw_sb[:, dc, vt, :cfg.VGW],
                            dc == 0, dc == DC - 1)
                ob = obp.tile([P, cfg.VPG, cfg.VGW], BF16, tag="ob")
                half = cfg.VPG // 2
                nc.vector.tensor_copy(ob[:, :half, :], po[:, :half, :cfg.VGW])
                nc.scalar.copy(ob[:, half:, :], po[:, half:, :cfg.VGW])
                nc.sync.dma_start(
                    out=out[tk * P:(tk + 1) * P, vg * GW:(vg + 1) * GW],
                    in_=ob)


# ---- host-side sharding / unsharding -----------------------------------

def _bf16(a):
    import ml_dtypes
    return np.ascontiguousarray(np.asarray(a, np.float32).astype(
        ml_dtypes.bfloat16))


def shard_inputs(cfg: Cfg, tokens, gate_signal, embed, pos_embed, rule_w1,
                 rule_b1, rule_w2, rule_b2, sel_w, sel_b, norm_g, norm_b,
                 lnf_g, lnf_b, head_w, n_cores=8):
    D, R, T = cfg.D, cfg.R, cfg.T
    w1t = _bf16(np.asarray(rule_w1, np.float32)
                .reshape(R, cfg.KC, P, cfg.HC, P).transpose(0, 3, 2, 1, 4))
    w2t = _bf16(np.asarray(rule_w2, np.float32)
                .reshape(R, cfg.HC, P, cfg.DC, P).transpose(0, 2, 1, 3, 4))
    shared = {
        "w1t": w1t,
        "b1": np.ascontiguousarray(rule_b1, np.float32),
        "w2t": w2t,
        "b2": _bf16(rule_b2),
        "ng": np.ascontiguousarray(norm_g, np.float32),
        "nb": np.ascontiguousarray(norm_b, np.float32),
        "lg": np.ascontiguousarray(lnf_g, np.float32).reshape(1, D),
        "lb": np.ascontiguousarray(lnf_b, np.float32).reshape(1, D),
        "headw": _bf16(head_w),
        "ones1": np.ones((P, 1), np.float32),
        "ones8d": np.ones((8, P), np.float32),
    }
    # rule selector: softmax(gate*scale @ sel_w + sel_b) -> [T, R]
    logits = (np.asarray(gate_signal, np.float32)[0] * cfg.gate_scale
              ) @ np.asarray(sel_w, np.float32) + np.asarray(sel_b, np.float32)
    logits -= logits.max(axis=-1, keepdims=True)
    e = np.exp(logits)
    rw = e / e.sum(axis=-1, keepdims=True)          # [T, R]
    embed = np.asarray(embed, np.float32)
    pos = np.asarray(pos_embed, np.float32)
    halves = T // cfg.own
    in_maps = []
    for c in range(n_cores):
        b, h = divmod(c, halves)
        t0 = h * cfg.own
        w = np.arange(t0 - cfg.halo, t0 - cfg.halo + cfg.WBUF) % T
        x0 = embed[np.asarray(tokens)[b, w]] + pos[w]      # [WBUF, D]
        x0t = np.ascontiguousarray(
            x0.T.reshape(cfg.DC, P, cfg.WBUF).transpose(1, 0, 2))
        m = dict(shared)
        m["xin"] = x0t
        m["xinb"] = _bf16(x0t)
        m["rwin"] = _bf16(rw[w, :].T)                      # [R, WBUF]
        in_maps.append(m)
    return in_maps


def unshard_output(cfg: Cfg, results, n_cores=8):
    halves = cfg.T // cfg.own
    out = np.empty((cfg.B, cfg.T, cfg.V), np.float32)
    for c in range(n_cores):
        b, h = divmod(c, halves)
        out[b, h * cfg.own:(h + 1) * cfg.own, :] = \
            np.asarray(results[c]["out"]).astype(np.float32)
    return out


_NC_CACHE = {}


def kernel(**inputs):
    cfg = Cfg()
    if "full" not in _NC_CACHE:
        _NC_CACHE["full"] = build_nc(cfg)
    nc = _NC_CACHE["full"]
    in_maps = shard_inputs(cfg, **{k: np.asarray(v) for k, v in inputs.items()})
    res = run_bass_kernel_spmd(nc, in_maps, core_ids=list(range(8)))
    return unshard_output(cfg, res.results)
